# revision 48
# baseline (speedup 1.0000x reference)
"""Trainium2 Bass kernel for nn_LonelyDecoder (dense transformer, 8-core TP).

Key observations baked in:
 - In the reference, every layer recomputes from the embedding output `h`
   and only the LAST layer's `out` feeds the logits -> layers 0..L-2 are
   dead code. We compute: embedding GEMM, layer L-1, output GEMM+softmax.
 - Scores are tiny ((q.k)/1024, |s| < ~1), so softmax needs no max pass.
 - Activations in the transformer body are TRANSPOSED ([feature, seq]);
   the output GEMM flips to [seq, vocab] so the softmax denominator
   falls out of the ACT accumulator and the scale is per-partition.
 - Collectives are chunked (bf16) and overlapped under compute.

Sharding (8 cores):
 - vocab dim of x/emb_W/out_W (4000/core, padded to 4096)
 - heads of attention (2 heads/core), DFF of the FFN (512/core)
"""

import numpy as np
import ml_dtypes

import concourse.bacc as bacc
import concourse.bass as bass
import concourse.mybir as mybir
import concourse.tile as tile
from concourse.bass_utils import run_bass_kernel_spmd

F32 = mybir.dt.float32
BF16 = mybir.dt.bfloat16
AF = mybir.ActivationFunctionType
ALU = mybir.AluOpType

S, V, D, H, DK, DFF, L = 2048, 32000, 1024, 16, 64, 4096, 4
NCORES = 8
VSR = V // NCORES          # 4000 real vocab shard
VSP = 4096                 # padded vocab shard (32 x 128)
NKC = VSP // 128           # 32 vocab k-chunks
NVC8 = VSP // 512          # 8 vocab chunks of 512
NDC = D // 128             # 8 d-chunks
NSC = 4                    # s-chunks of 512
SC = 512
NTT = S // 128             # 16 t-tiles
FS = DFF // NCORES         # 512 ff shard
NFC = FS // 128            # 4 ff chunks
RG = [list(range(NCORES))]

LAST_RESULTS = {}          # stash for test harness (exec time etc.)


def ts(i, n):
    return slice(i * n, (i + 1) * n)


def build_bass():
    nc = bacc.Bacc(None, target_bir_lowering=False)

    # ---- I/O ----
    xT = nc.dram_tensor("xT", [VSP, S], BF16, kind="ExternalInput")
    embW = nc.dram_tensor("embW", [VSP, D], BF16, kind="ExternalInput")
    pebT = nc.dram_tensor("pebT", [D, S], BF16, kind="ExternalInput")
    qkw = [nc.dram_tensor(f"qkw{m}", [128, NDC, 256], BF16, kind="ExternalInput") for m in (1, 2)]
    bqk = [nc.dram_tensor(f"bqk{m}", [128, 2], F32, kind="ExternalInput") for m in (1, 2)]
    vw = [nc.dram_tensor(f"vw{m}", [128, NDC, 128], BF16, kind="ExternalInput") for m in (1, 2)]
    identT = nc.dram_tensor("identT", [128, 128], BF16, kind="ExternalInput")
    bv = [nc.dram_tensor(f"bv{m}", [128, 1], F32, kind="ExternalInput") for m in (1, 2)]
    maskT = nc.dram_tensor("maskT", [128, 4 * SC], BF16, kind="ExternalInput")
    f1w = nc.dram_tensor("f1w", [128, NDC, FS], BF16, kind="ExternalInput")
    f1b = nc.dram_tensor("f1b", [128, NFC], F32, kind="ExternalInput")
    f2w = nc.dram_tensor("f2w", [128, NFC, D], BF16, kind="ExternalInput")
    f2bT = nc.dram_tensor("f2bT", [128, NDC], F32, kind="ExternalInput")
    lngT = nc.dram_tensor("lngT", [128, NDC], F32, kind="ExternalInput")
    lnbT = nc.dram_tensor("lnbT", [128, NDC], F32, kind="ExternalInput")
    outw = nc.dram_tensor("outw", [128, NDC, VSP], BF16, kind="ExternalInput")
    outbV = nc.dram_tensor("outbV", [1, VSP], BF16, kind="ExternalInput")
    probsO = nc.dram_tensor("probsO", [S, VSP], F32, kind="ExternalOutput")
    import os as _os
    DBG = bool(int(_os.environ.get("KB_DEBUG", "0")))
    if DBG:
        dbg_hT = nc.dram_tensor("dbg_hT", [128, NDC, S], BF16, kind="ExternalOutput")
        dbg_h1T = nc.dram_tensor("dbg_h1T", [128, NDC, S], BF16, kind="ExternalOutput")
        dbg_h2T = nc.dram_tensor("dbg_h2T", [128, NDC, S], BF16, kind="ExternalOutput")
        dbg_outT = nc.dram_tensor("dbg_outT", [128, NDC, S], BF16, kind="ExternalOutput")
        dbg_at1 = nc.dram_tensor("dbg_at1", [128, NSC, SC], BF16, kind="ExternalOutput")
        dbg_at2 = nc.dram_tensor("dbg_at2", [128, NSC, SC], BF16, kind="ExternalOutput")
        dbg_e = nc.dram_tensor("dbg_e", [128, VSP], BF16, kind="ExternalOutput")
        dbg_den = nc.dram_tensor("dbg_den", [128, NVC8], F32, kind="ExternalOutput")
        dbg_hpar = nc.dram_tensor("dbg_hpar", [D, S], BF16, kind="ExternalOutput")
        dbg_hred = nc.dram_tensor("dbg_hred", [D, S], BF16, kind="ExternalOutput")

    with tile.TileContext(nc) as tc:
        with tc.tile_pool(name="dram", bufs=1, space="DRAM") as dram, \
             tc.tile_pool(name="ps", bufs=3, space="PSUM") as ps, \
             tc.tile_pool(name="ps2", bufs=2, space="PSUM") as ps2, \
             tc.tile_pool(name="const", bufs=1) as const, \
             tc.tile_pool(name="ev", bufs=2) as evp, \
             tc.tile_pool(name="addin", bufs=3) as adp, \
             tc.tile_pool(name="addin2", bufs=1) as adp2, \
             tc.tile_pool(name="x2p", bufs=2) as x2p, \
             tc.tile_pool(name="bcast", bufs=2) as bcp, \
             tc.tile_pool(name="small", bufs=2) as smp:

            # internal DRAM (tracked pool tiles)
            h_par = [dram.tile([D, 2 * SC], BF16, tag=f"hp{p}", name=f"h_par{p}")
                     for p in range(2)]
            h_red = [dram.tile([D, 2 * SC], BF16, tag=f"hr{p}", addr_space="Shared",
                               name=f"h_red{p}") for p in range(2)]
            a_in = [[dram.tile([128, SC], BF16, tag=f"ai{m}_{sc}", name=f"a{m}_in{sc}")
                     for sc in range(NSC)] for m in (0, 1)]
            a_out = [[dram.tile([D, SC], BF16, tag=f"ao{m}_{sc}", addr_space="Shared",
                                name=f"a{m}_out{sc}") for sc in range(NSC)] for m in (0, 1)]
            y_par = [dram.tile([D, SC], BF16, tag=f"yp{sc}", name=f"y_par{sc}")
                     for sc in range(NSC)]
            y_red = [dram.tile([D, SC], BF16, tag=f"yr{sc}", addr_space="Shared",
                               name=f"y_red{sc}") for sc in range(NSC)]
            ss_in = [dram.tile([128, 1], F32, tag=f"si{tt}", name=f"ss_in{tt}")
                     for tt in range(NTT)]
            ss_out = [dram.tile([128, 1], F32, tag=f"so{tt}", addr_space="Shared",
                                name=f"ss_out{tt}") for tt in range(NTT)]

            # constants
            ones_bf_col = const.tile([128, 1], BF16, tag="c1")
            nc.vector.memset(ones_bf_col[:, :], 1.0)
            ones_row = const.tile([1, 128], F32, tag="c3")
            nc.vector.memset(ones_row[:, :], 1.0)
            ones_row64 = const.tile([1, 64], F32, tag="c4")
            nc.vector.memset(ones_row64[:, :], 1.0)
            ones_row128_bf = const.tile([1, 128], BF16, tag="c6")
            nc.vector.memset(ones_row128_bf[:, :], 1.0)
            eps_tile = const.tile([1, 1], F32, tag="c5")
            nc.vector.memset(eps_tile[:, :], 1e-5)
            ident_sb = const.tile([128, 128], BF16, tag="ident")
            nc.sync.dma_start(ident_sb[:, :], identT[:, :])
            bqk_sb = [const.tile([128, 2], F32, tag=f"bqk{m}", name=f"bqk_sb{m}") for m in range(2)]
            bv_sb = [const.tile([128, 1], F32, tag=f"bv{m}", name=f"bv_sb{m}") for m in range(2)]
            for m in range(2):
                nc.sync.dma_start(bqk_sb[m][:, :], bqk[m][:, :])
                nc.sync.dma_start(bv_sb[m][:, :], bv[m][:, :])
            f1b_sb = const.tile([128, NFC], F32, tag="f1b")
            nc.sync.dma_start(f1b_sb[:, :], f1b[:, :])
            f2bT_sb = const.tile([128, NDC], F32, tag="f2bT")
            nc.sync.dma_start(f2bT_sb[:, :], f2bT[:, :])
            lng_sb = const.tile([128, NDC], F32, tag="lng")
            nc.sync.dma_start(lng_sb[:, :], lngT[:, :])
            lnb_sb = const.tile([128, NDC], F32, tag="lnb")
            nc.sync.dma_start(lnb_sb[:, :], lnbT[:, :])

            # ---------- embedding GEMM:  hT_partial = embW^T @ xT ----------
            # sc-pairs of 1024 tokens; dc-pair groups (4 PSUM banks) so
            # eviction of one group overlaps the next; xT half resident.
            with tc.tile_pool(name="embw", bufs=1) as embp, \
                 tc.tile_pool(name="xt", bufs=1) as xtp:
                embw_sb = embp.tile([128, NKC, D], BF16, tag="embw")
                xthalf = xtp.tile([128, NKC, 2 * SC], BF16, tag="xt")
                for p in range(2):
                    for kc in range(NKC):
                        if p == 0:
                            nc.sync.dma_start(embw_sb[:, kc, :], embW[ts(kc, 128), :])
                        nc.sync.dma_start(
                            xthalf[:, kc, :], xT[ts(kc, 128), ts(p, 2 * SC)])
                    for dc in range(NDC):
                        pes = ps2.tile([128, 2, SC], F32, tag="ps2",
                                       name=f"pe_{p}_{dc}")
                        for kc in range(NKC):
                            for hh in range(2):
                                nc.tensor.matmul(
                                    pes[:, hh, :],
                                    embw_sb[:, kc, ts(dc, 128)],
                                    xthalf[:, kc, ts(hh, SC)],
                                    start=(kc == 0),
                                    stop=(kc == NKC - 1),
                                )
                        hv = evp.tile([128, 2 * SC], BF16, tag="ev")
                        nc.scalar.activation(hv[:, :], pes[:, :, :], AF.Copy)
                        nc.sync.dma_start(h_par[p][ts(dc, 128), :], hv[:, :])
                    nc.gpsimd.collective_compute(
                        "AllReduce", ALU.add, replica_groups=RG,
                        ins=[h_par[p][:, :].opt()], outs=[h_red[p][:, :].opt()],
                    )
                    if DBG:
                        nc.sync.dma_start(dbg_hpar[:, ts(p, 2 * SC)], h_par[p][:, :])
                        nc.sync.dma_start(dbg_hred[:, ts(p, 2 * SC)], h_red[p][:, :])

            # ======== transformer body (s-chunk pipelined) ========
            with tc.tile_pool(name="acts", bufs=2) as acts:
                # residual + layernorm over feature dim for ONE s-chunk.
                # residual sum written IN PLACE into prevT (dead after).
                def ln_sc(prevT, sc, addin_dram, newT, name, extra_bias=None):
                    scs = ts(sc, SC)
                    for dc in range(NDC):
                        ad = adp.tile([128, SC], BF16, tag="addin",
                                      name=f"ad_{name}_{dc}")
                        nc.sync.dma_start(ad[:, :], addin_dram[ts(dc, 128), :])
                        if extra_bias is not None:
                            ab = adp2.tile([128, SC], BF16, tag="addin2",
                                           name=f"ab_{name}_{dc}")
                            nc.scalar.activation(ab[:, :], ad[:, :], AF.Identity,
                                                 bias=extra_bias[:, dc:dc + 1])
                            nc.vector.tensor_add(prevT[:, dc, scs], prevT[:, dc, scs], ab[:, :])
                        else:
                            nc.vector.tensor_add(prevT[:, dc, scs], prevT[:, dc, scs], ad[:, :])
                    st = ps.tile([65, SC], F32, tag="ps", name=f"st_{name}")
                    for dc in range(NDC):
                        x2 = x2p.tile([128, SC], BF16, tag="x2", name=f"x2_{name}_{dc}")
                        nc.vector.tensor_mul(x2[:, :], prevT[:, dc, scs], prevT[:, dc, scs])
                        nc.tensor.matmul(st[0:1, :], ones_bf_col[:, :],
                                         prevT[:, dc, scs],
                                         start=(dc == 0), stop=(dc == NDC - 1))
                        nc.tensor.matmul(st[64:65, :], ones_bf_col[:, :],
                                         x2[:, :],
                                         start=(dc == 0), stop=(dc == NDC - 1))
                    nm = smp.tile([1, SC], BF16, tag="nm", name=f"nm_{name}")
                    nc.vector.tensor_scalar_mul(nm[:, :], st[0:1, :], -1.0 / D)
                    e2 = smp.tile([1, SC], F32, tag="e2", name=f"e2_{name}")
                    nc.vector.tensor_scalar_mul(e2[:, :], st[64:65, :], 1.0 / D)
                    musq = smp.tile([1, SC], F32, tag="musq", name=f"musq_{name}")
                    nc.vector.tensor_mul(musq[:, :], nm[:, :], nm[:, :])
                    nc.vector.tensor_sub(e2[:, :], e2[:, :], musq[:, :])
                    nc.scalar.activation(e2[:, :], e2[:, :], AF.Sqrt,
                                         bias=eps_tile[:, :])
                    inv = smp.tile([1, SC], BF16, tag="inv1", name=f"inv_{name}")
                    with nc.allow_low_precision(reason="bf16 LN scale is within tolerance"):
                        nc.vector.reciprocal(inv[:, :], e2[:, :])
                    negmu_b = bcp.tile([128, SC], BF16, tag="negmu", bufs=1,
                                       name=f"nmb_{name}")
                    inv_b = bcp.tile([128, SC], BF16, tag="inv", bufs=1,
                                     name=f"invb_{name}")
                    nc.gpsimd.partition_broadcast(negmu_b[:, :], nm[:, :])
                    nc.gpsimd.partition_broadcast(inv_b[:, :], inv[:, :])
                    for dc in range(NDC):
                        t1 = x2p.tile([128, SC], BF16, tag="t1b", name=f"t1_{name}_{dc}")
                        nc.vector.tensor_add(t1[:, :], prevT[:, dc, scs], negmu_b[:, :])
                        nc.vector.tensor_mul(t1[:, :], t1[:, :], inv_b[:, :])
                        nc.vector.tensor_scalar(newT[:, dc, scs], t1[:, :],
                                                lng_sb[:, dc:dc + 1],
                                                lnb_sb[:, dc:dc + 1],
                                                op0=ALU.mult, op1=ALU.add)

                hT = acts.tile([128, NDC, S], BF16, tag="act", name="hT")

                def build_hT(p):
                    for dc in range(NDC):
                        hr = adp.tile([128, 2 * SC], BF16, tag="hrad", bufs=2,
                                      name=f"hr_{p}_{dc}")
                        nc.sync.dma_start(hr[:, :], h_red[p][ts(dc, 128), :])
                        pb = adp.tile([128, 2 * SC], BF16, tag="pbad", bufs=2,
                                      name=f"pb_{p}_{dc}")
                        nc.sync.dma_start(pb[:, :], pebT[ts(dc, 128), ts(p, 2 * SC)])
                        nc.vector.tensor_add(hT[:, dc, ts(p, 2 * SC)], hr[:, :], pb[:, :])

                def qkv_sc(mi, actT, sc, qT2, kT2, V_sb, qkw_sb, vw_sb):
                    """Q/K projections + V~ tiles for one s-chunk.
                    V is computed weight-stationary ([dk2, t] in PSUM), then
                    PE-transposed per t-tile into the [t, V_h0|1|V_h1|1]
                    layout the AV matmul wants."""
                    scs = ts(sc, SC)
                    for wi, dst in ((0, qT2), (1, kT2)):
                        pq = ps.tile([128, SC], F32, tag="ps",
                                     name=f"pq{mi}_{wi}_{sc}")
                        for dc in range(NDC):
                            nc.tensor.matmul(
                                pq[:, :],
                                qkw_sb[:, dc, ts(wi, 128)],
                                actT[:, dc, scs],
                                start=(dc == 0), stop=(dc == NDC - 1),
                            )
                        nc.vector.tensor_scalar_add(
                            dst[:, sc, :], pq[:, :], bqk_sb[mi][:, wi:wi + 1])
                    pvt = ps.tile([128, SC], F32, tag="ps", name=f"pvt{mi}_{sc}")
                    for dc in range(NDC):
                        nc.tensor.matmul(
                            pvt[:, :], vw_sb[:, dc, :], actT[:, dc, scs],
                            start=(dc == 0), stop=(dc == NDC - 1),
                        )
                    VT_tmp = x2p.tile([128, SC], BF16, tag="vtt",
                                      name=f"vtt{mi}_{sc}")
                    nc.scalar.activation(VT_tmp[:, :], pvt[:, :], AF.Copy)
                    for ttl in range(4):
                        tt = 4 * sc + ttl
                        ptr = ps.tile([128, 128], BF16, tag="ptr", bufs=1,
                                      name=f"ptr{mi}_{tt}")
                        nc.tensor.transpose(ptr[:, :], VT_tmp[:, ts(ttl, 128)],
                                            ident_sb[:, :])
                        nc.vector.tensor_copy(V_sb[:, tt, 0:64], ptr[:, 0:64])
                        nc.vector.tensor_copy(V_sb[:, tt, 65:129], ptr[:, 64:128])
                        nc.vector.memset(V_sb[:, tt, 64:65], 1.0)
                        nc.vector.memset(V_sb[:, tt, 129:130], 1.0)

                def scores_sc(mi, sc, qT2, kT2, V_sb, mask_sb, attnT):
                    masked = mask_sb is not None
                    """Packed-head scores + AV for one s-chunk; writes
                    normalized attn (+bias) into attnT[:, sc-slice],
                    DMAs to a_in and AllGathers to a_out."""
                    po = [ps.tile([128, SC], F32, tag="ps", name=f"po{mi}_{h}_{sc}")
                          for h in range(2)]
                    tts = list(range(4 * (sc + 1))) if masked else list(range(NTT))
                    for i, tt in enumerate(tts):
                        pscr = ps2.tile([128, 2, SC], F32, tag="ps2",
                                        name=f"pscr{mi}_{sc}_{tt}")
                        for h in range(2):
                            nc.tensor.matmul(
                                pscr[:, h, :],
                                kT2[ts(h, 64), tt // 4, ts(tt % 4, 128)],
                                qT2[ts(h, 64), sc, :],
                                start=True, stop=True,
                            )
                        et = evp.tile([128, 2, SC], BF16, tag="exp")
                        nc.scalar.activation(et[:, :, :], pscr[:, :, :], AF.Exp,
                                             scale=1.0 / D)
                        if masked and tt >= 4 * sc:
                            for h in range(2):
                                nc.vector.tensor_mul(
                                    et[:, h, :], et[:, h, :],
                                    mask_sb[:, ts(tt - 4 * sc, SC)],
                                )
                        for h in range(2):
                            nc.tensor.matmul(
                                po[h][0:65, :],
                                V_sb[:, tt, ts(h, 65)],
                                et[:, h, :],
                                start=(i == 0), stop=(i == len(tts) - 1),
                            )
                    for h in range(2):
                        rec = smp.tile([1, SC], BF16, tag="rec", bufs=2,
                                       name=f"rec{mi}_{h}_{sc}")
                        with nc.allow_low_precision(reason="bf16 softmax scale is within tolerance"):
                            nc.vector.reciprocal(rec[:, :], po[h][64:65, :])
                        rb = bcp.tile([64, SC], BF16, tag="rb", bufs=2,
                                      name=f"rb{mi}_{h}_{sc}")
                        nc.gpsimd.partition_broadcast(rb[:, :], rec[:, :])
                        tmp = x2p.tile([128, SC], BF16, tag="t1b",
                                       name=f"avtmp{mi}_{h}_{sc}")
                        nc.vector.tensor_mul(tmp[0:64, :], po[h][0:64, :], rb[:, :])
                        nc.vector.tensor_scalar_add(
                            attnT[ts(h, 64), sc, :], tmp[0:64, :],
                            bv_sb[mi][ts(h, 64), :])
                    nc.sync.dma_start(a_in[mi][sc][:, :], attnT[:, sc, :])
                    nc.gpsimd.collective_compute(
                        "AllGather", ALU.bypass, replica_groups=RG,
                        ins=[a_in[mi][sc][:, :].opt()], outs=[a_out[mi][sc][:, :].opt()],
                    )

                # ---- MHA1 (masked), pipelined over sc as AR chunks land
                with tc.tile_pool(name="aw1", bufs=1) as aw1, \
                     tc.tile_pool(name="at1", bufs=1) as at1:
                    qkw1_sb = aw1.tile([128, NDC, 256], BF16, tag="qkw1")
                    nc.sync.dma_start(qkw1_sb[:, :, :], qkw[0][:, :, :])
                    vw1_sb = aw1.tile([128, NDC, 128], BF16, tag="vw1")
                    nc.sync.dma_start(vw1_sb[:, :, :], vw[0][:, :, :])
                    mask_sb = at1.tile([128, 4 * SC], BF16, tag="mask")
                    nc.sync.dma_start(mask_sb[:, :], maskT[:, :])
                    qT2_1 = at1.tile([128, NSC, SC], BF16, tag="qT1", name="qT2_1")
                    kT2_1 = at1.tile([128, NSC, SC], BF16, tag="kT1", name="kT2_1")
                    V_sb1 = at1.tile([128, NTT, 130], BF16, tag="V1", name="V_sb1")
                    attnT1 = at1.tile([128, NSC, SC], BF16, tag="atn1", name="attnT1")
                    for p in range(2):
                        build_hT(p)
                        for sc in (2 * p, 2 * p + 1):
                            qkv_sc(0, hT, sc, qT2_1, kT2_1, V_sb1, qkw1_sb, vw1_sb)
                            scores_sc(0, sc, qT2_1, kT2_1, V_sb1, mask_sb, attnT1)
                    if DBG:
                        nc.sync.dma_start(dbg_hT[:, :, :], hT[:, :, :])
                        nc.sync.dma_start(dbg_at1[:, :, :], attnT1[:, :, :])

                # ---- h1 = LN(h + attn1) per sc; QKV2 per sc; MHA2 scores
                h1T = acts.tile([128, NDC, S], BF16, tag="act", name="h1T")
                with tc.tile_pool(name="aw2", bufs=1) as aw2, \
                     tc.tile_pool(name="at2", bufs=1) as at2:
                    qkw2_sb = aw2.tile([128, NDC, 256], BF16, tag="qkw2")
                    nc.sync.dma_start(qkw2_sb[:, :, :], qkw[1][:, :, :])
                    vw2_sb = aw2.tile([128, NDC, 128], BF16, tag="vw2")
                    nc.sync.dma_start(vw2_sb[:, :, :], vw[1][:, :, :])
                    qT2_2 = at2.tile([128, NSC, SC], BF16, tag="qT2", name="qT2_2")
                    kT2_2 = at2.tile([128, NSC, SC], BF16, tag="kT2", name="kT2_2")
                    V_sb2 = at2.tile([128, NTT, 130], BF16, tag="V2", name="V_sb2")
                    attnT2 = at2.tile([128, NSC, SC], BF16, tag="atn2", name="attnT2")
                    for sc in range(NSC):
                        ln_sc(hT, sc, a_out[0][sc], h1T, f"h1_{sc}")
                        qkv_sc(1, h1T, sc, qT2_2, kT2_2, V_sb2, qkw2_sb, vw2_sb)
                    for sc in range(NSC):
                        scores_sc(1, sc, qT2_2, kT2_2, V_sb2, None, attnT2)
                    if DBG:
                        nc.sync.dma_start(dbg_h1T[:, :, :], h1T[:, :, :])
                        nc.sync.dma_start(dbg_at2[:, :, :], attnT2[:, :, :])

                # ---- h2 = LN(h1 + attn2) per sc; FFN per sc; ARy per sc
                h2T = acts.tile([128, NDC, S], BF16, tag="act", name="h2T")
                with tc.tile_pool(name="outwp", bufs=1) as owp:
                    # outw prefetch: DMAs overlap the whole FFN phase
                    outw_sb = owp.tile([128, NDC, VSP], BF16, tag="outw")
                    for dc in range(NDC):
                        nc.sync.dma_start(outw_sb[:, dc, :], outw[:, dc, :])
                    outb_sb = owp.tile([1, VSP], BF16, tag="outb")
                    nc.sync.dma_start(outb_sb[:, :], outbV[:, :])
                    with tc.tile_pool(name="ffw", bufs=1) as ffp:
                        f1w_sb = ffp.tile([128, NDC, FS], BF16, tag="f1w")
                        nc.sync.dma_start(f1w_sb[:, :, :], f1w[:, :, :])
                        f2w_sb = ffp.tile([128, NFC, D], BF16, tag="f2w")
                        nc.sync.dma_start(f2w_sb[:, :, :], f2w[:, :, :])
                        for sc in range(NSC):
                            ln_sc(h1T, sc, a_out[1][sc], h2T, f"h2_{sc}")
                            uT = ffp.tile([128, NFC, SC], BF16, tag="uT", bufs=2,
                                          name=f"uT_{sc}")
                            for fc in range(NFC):
                                pu = ps.tile([128, SC], F32, tag="ps", name=f"pu_{fc}_{sc}")
                                for dc in range(NDC):
                                    nc.tensor.matmul(pu[:, :], f1w_sb[:, dc, ts(fc, 128)],
                                                     h2T[:, dc, ts(sc, SC)],
                                                     start=(dc == 0), stop=(dc == NDC - 1))
                                nc.scalar.activation(uT[:, fc, :], pu[:, :], AF.Relu,
                                                     bias=f1b_sb[:, fc:fc + 1])
                            for dc in range(NDC):
                                py = ps.tile([128, SC], F32, tag="ps", name=f"py_{dc}_{sc}")
                                for fc in range(NFC):
                                    nc.tensor.matmul(py[:, :], f2w_sb[:, fc, ts(dc, 128)],
                                                     uT[:, fc, :],
                                                     start=(fc == 0), stop=(fc == NFC - 1))
                                yt = evp.tile([128, SC], BF16, tag="ev")
                                nc.scalar.activation(yt[:, :], py[:, :], AF.Copy)
                                nc.sync.dma_start(y_par[sc][ts(dc, 128), :], yt[:, :])
                            nc.gpsimd.collective_compute(
                                "AllReduce", ALU.add, replica_groups=RG,
                                ins=[y_par[sc][:, :].opt()], outs=[y_red[sc][:, :].opt()],
                            )

                    # ==== out = LN(h2+ffn); logits GEMM + fused softmax ====
                    # [token, vocab] layout: stationary outT tile reused over
                    # vocab chunks, exp eviction accumulates the denominator.
                    outT = acts.tile([128, NDC, S], BF16, tag="act", name="outT")
                    if DBG:
                        nc.sync.dma_start(dbg_h2T[:, :, :], h2T[:, :, :])
                    with tc.tile_pool(name="esb", bufs=2) as esp, \
                         tc.tile_pool(name="den", bufs=2) as denp, \
                         tc.tile_pool(name="pp", bufs=3) as ppp:
                        for sc in range(NSC):
                            ln_sc(h2T, sc, y_red[sc], outT, f"out_{sc}",
                                  extra_bias=f2bT_sb)
                            for ttl in range(4):
                                tt = 4 * sc + ttl
                                e_sb = esp.tile([128, VSP], BF16, tag="esb",
                                                name=f"esb_{tt}")
                                den = denp.tile([128, 4], F32, tag="den",
                                                name=f"den_{tt}")
                                den1 = denp.tile([128, 1], F32, tag="den1",
                                                 name=f"den1_{tt}")
                                for vq in range(4):      # vocab quarters: 2 banks
                                    pls = ps2.tile([128, 2, SC], F32, tag="ps2",
                                                   name=f"pl_{tt}_{vq}")
                                    for i in range(2):
                                        vc = vq * 2 + i
                                        nc.tensor.matmul(pls[:, i, :], ones_row128_bf[:, :],
                                                         outb_sb[:, ts(vc, SC)],
                                                         start=True, stop=False)
                                    for dc in range(NDC):
                                        for i in range(2):
                                            vc = vq * 2 + i
                                            nc.tensor.matmul(
                                                pls[:, i, :],
                                                outT[:, dc, ts(tt, 128)],
                                                outw_sb[:, dc, ts(vc, SC)],
                                                start=False, stop=(dc == NDC - 1),
                                            )
                                    nc.scalar.activation(
                                        e_sb[:, ts(vq, 2 * SC)], pls[:, :, :], AF.Exp,
                                        accum_out=den[:, vq:vq + 1],
                                    )
                                nc.vector.tensor_reduce(
                                    den1[:, :], den[:, :],
                                    axis=mybir.AxisListType.X, op=ALU.add,
                                )
                                if DBG and tt == 0:
                                    nc.sync.dma_start(dbg_e[:, :], e_sb[:, :])
                                    nc.sync.dma_start(dbg_den[:, :], den[:, :])
                                nc.sync.dma_start(ss_in[tt][:, :], den1[:, :])
                                nc.gpsimd.collective_compute(
                                    "AllReduce", ALU.add, replica_groups=RG,
                                    ins=[ss_in[tt][:, :].opt()], outs=[ss_out[tt][:, :].opt()],
                                )
                                ssum = denp.tile([128, 1], F32, tag="ssum",
                                                 name=f"ssum_{tt}")
                                nc.sync.dma_start(ssum[:, :], ss_out[tt][:, :])
                                srec = denp.tile([128, 1], F32, tag="srec",
                                                 name=f"srec_{tt}")
                                nc.vector.reciprocal(srec[:, :], ssum[:, :])
                                for vc in range(NVC8):
                                    pr = ppp.tile([128, SC], F32, tag="pp",
                                                  name=f"pp_{tt}_{vc}")
                                    nc.gpsimd.tensor_scalar_mul(
                                        pr[:, :], e_sb[:, ts(vc, SC)],
                                        srec[:, 0:1])
                                    nc.sync.dma_start(
                                        probsO[ts(tt, 128), ts(vc, SC)], pr[:, :])
                        if DBG:
                            nc.sync.dma_start(dbg_outT[:, :, :], outT[:, :, :])

    nc.compile()
    return nc


def _positional_encoding():
    pos = np.arange(S, dtype=np.float32)[:, None]
    i = np.arange(0, D, 2, dtype=np.float32)
    ang = (pos * np.exp((-np.log(10000.0) * i / D).astype(np.float32))).astype(np.float32)
    pe = np.zeros((S, D), np.float32)
    pe[:, 0::2] = np.sin(ang)
    pe[:, 1::2] = np.cos(ang)
    return pe


def _bf(x):
    return np.ascontiguousarray(x).astype(ml_dtypes.bfloat16)


def _f32(x):
    return np.ascontiguousarray(x, dtype=np.float32)


def prepare_inputs(inp):
    """Full fp32 inputs -> per-core input maps (host-side sharding/layout)."""
    li = L - 1
    xT_full = np.ascontiguousarray(inp["x"].T)          # [V, S]
    peb = (inp["emb_b"][None, :] + _positional_encoding()).astype(np.float32)
    pebT = _bf(peb.T)                                    # [D, S]

    # causal mask patterns for the 4 diagonal t-tiles of an s-chunk
    t_loc = np.arange(128)[:, None]
    s_loc = np.arange(SC)[None, :]
    maskT = np.concatenate(
        [((p * 128 + t_loc) <= s_loc).astype(np.float32) for p in range(4)], axis=1
    )
    maskT = _bf(maskT)                                   # [128, 2048]

    in_maps = []
    for c in range(NCORES):
        m = {}
        xs = xT_full[c * VSR:(c + 1) * VSR]              # [4000, S]
        m["xT"] = _bf(np.concatenate([xs, np.zeros((VSP - VSR, S), np.float32)], 0))
        ew = inp["emb_W"][c * VSR:(c + 1) * VSR]
        m["embW"] = _bf(np.concatenate([ew, np.zeros((VSP - VSR, D), np.float32)], 0))
        m["pebT"] = pebT
        m["maskT"] = maskT
        m["identT"] = _bf(np.eye(128, dtype=np.float32))
        for mi, (Wq, bq, Wk, bk, Wv, bvv) in enumerate([
            (inp["Wq1"][li], inp["bq1"][li], inp["Wk1"][li], inp["bk1"][li],
             inp["Wv1"][li], inp["bv1"][li]),
            (inp["Wq2"][li], inp["bq2"][li], inp["Wk2"][li], inp["bk2"][li],
             inp["Wv2"][li], inp["bv2"][li]),
        ]):
            h0, h1 = 2 * c, 2 * c + 1
            qk = np.concatenate([Wq[h0], Wq[h1], Wk[h0], Wk[h1]], axis=1)  # [D, 256]
            m[f"qkw{mi+1}"] = _bf(qk.reshape(NDC, 128, 256).transpose(1, 0, 2))
            m[f"bqk{mi+1}"] = _f32(np.stack(
                [np.concatenate([bq[h0], bq[h1]]),
                 np.concatenate([bk[h0], bk[h1]])], axis=1))
            vp = np.concatenate([Wv[h0], Wv[h1]], axis=1)   # [D, 128]
            m[f"vw{mi+1}"] = _bf(vp.reshape(NDC, 128, 128).transpose(1, 0, 2))
            m[f"bv{mi+1}"] = _f32(np.concatenate([bvv[h0], bvv[h1]])[:, None])
        w1 = inp["ff_W1"][li][:, c * FS:(c + 1) * FS]    # [D, FS]
        m["f1w"] = _bf(w1.reshape(NDC, 128, FS).transpose(1, 0, 2))
        m["f1b"] = _f32(inp["ff_b1"][li][c * FS:(c + 1) * FS].reshape(NFC, 128).T)
        w2 = inp["ff_W2"][li][c * FS:(c + 1) * FS]       # [FS, D]
        m["f2w"] = _bf(w2.reshape(NFC, 128, D).transpose(1, 0, 2))
        m["f2bT"] = _f32(inp["ff_b2"][li].reshape(NDC, 128).T)
        m["lngT"] = _f32(inp["ln_g"].reshape(NDC, 128).T)
        m["lnbT"] = _f32(inp["ln_b"].reshape(NDC, 128).T)
        ow = inp["out_W"][:, c * VSR:(c + 1) * VSR]      # [D, 4000]
        ow = np.concatenate([ow, np.zeros((D, VSP - VSR), np.float32)], axis=1)
        m["outw"] = _bf(ow.reshape(NDC, 128, VSP).transpose(1, 0, 2))
        ob = np.full(VSP, -30.0, np.float32)
        ob[:VSR] = inp["out_b"][c * VSR:(c + 1) * VSR]
        m["outbV"] = _bf(ob[None, :])
        in_maps.append(m)
    return in_maps


_NC_CACHE = {}


def kernel(**inputs):
    inputs = {k: np.asarray(v, dtype=np.float32) for k, v in inputs.items()}
    if "nc" not in _NC_CACHE:
        _NC_CACHE["nc"] = build_bass()
    nc = _NC_CACHE["nc"]
    in_maps = prepare_inputs(inputs)
    import os
    trace = bool(int(os.environ.get("KB_TRACE", "0")))
    res = run_bass_kernel_spmd(nc, in_maps, list(range(NCORES)), trace=trace)
    LAST_RESULTS["res"] = res
    shards = [res.results[c]["probsO"][:, :VSR] for c in range(NCORES)]
    return np.ascontiguousarray(np.concatenate(shards, axis=1))


# revision 49
# speedup vs baseline: 1.4753x; 1.4753x over previous
"""Trainium2 Bass kernel for nn_LonelyDecoder (dense transformer, 8-core TP).

Key observations baked in:
 - In the reference, every layer recomputes from the embedding output `h`
   and only the LAST layer's `out` feeds the logits -> layers 0..L-2 are
   dead code. We compute: embedding GEMM, layer L-1, output GEMM+softmax.
 - Scores are tiny ((q.k)/1024, |s| < ~1), so softmax needs no max pass.
 - Activations in the transformer body are TRANSPOSED ([feature, seq]);
   the output GEMM flips to [seq, vocab] so the softmax denominator
   falls out of the ACT accumulator and the scale is per-partition.
 - Collectives are chunked (bf16) and overlapped under compute.

Sharding (8 cores):
 - vocab dim of x/emb_W/out_W (4000/core, padded to 4096)
 - heads of attention (2 heads/core), DFF of the FFN (512/core)
"""

import numpy as np
import ml_dtypes

import concourse.bacc as bacc
import concourse.bass as bass
import concourse.mybir as mybir
import concourse.tile as tile
from concourse.bass_utils import run_bass_kernel_spmd

F32 = mybir.dt.float32
BF16 = mybir.dt.bfloat16
AF = mybir.ActivationFunctionType
ALU = mybir.AluOpType

S, V, D, H, DK, DFF, L = 2048, 32000, 1024, 16, 64, 4096, 4
NCORES = 8
VSR = V // NCORES          # 4000 real vocab shard
VSP = 4096                 # padded vocab shard (32 x 128)
NKC = VSP // 128           # 32 vocab k-chunks
NVC8 = VSP // 512          # 8 vocab chunks of 512
NDC = D // 128             # 8 d-chunks
NSC = 4                    # s-chunks of 512
SC = 512
NTT = S // 128             # 16 t-tiles
FS = DFF // NCORES         # 512 ff shard
NFC = FS // 128            # 4 ff chunks
RG = [list(range(NCORES))]

LAST_RESULTS = {}          # stash for test harness (exec time etc.)


def ts(i, n):
    return slice(i * n, (i + 1) * n)


def build_bass():
    nc = bacc.Bacc(None, target_bir_lowering=False)

    # ---- I/O ----
    xT = nc.dram_tensor("xT", [VSP, S], BF16, kind="ExternalInput")
    embW = nc.dram_tensor("embW", [VSP, D], BF16, kind="ExternalInput")
    pebT = nc.dram_tensor("pebT", [D, S], BF16, kind="ExternalInput")
    qkw = [nc.dram_tensor(f"qkw{m}", [128, NDC, 256], BF16, kind="ExternalInput") for m in (1, 2)]
    bqk = [nc.dram_tensor(f"bqk{m}", [128, 2], F32, kind="ExternalInput") for m in (1, 2)]
    vw = [nc.dram_tensor(f"vw{m}", [128, NDC, 128], BF16, kind="ExternalInput") for m in (1, 2)]
    identT = nc.dram_tensor("identT", [128, 128], BF16, kind="ExternalInput")
    bv = [nc.dram_tensor(f"bv{m}", [128, 1], F32, kind="ExternalInput") for m in (1, 2)]
    maskT = nc.dram_tensor("maskT", [128, 4 * SC], BF16, kind="ExternalInput")
    f1w = nc.dram_tensor("f1w", [128, NDC, FS], BF16, kind="ExternalInput")
    f1b = nc.dram_tensor("f1b", [128, NFC], F32, kind="ExternalInput")
    f2w = nc.dram_tensor("f2w", [128, NFC, D], BF16, kind="ExternalInput")
    f2bT = nc.dram_tensor("f2bT", [128, NDC], F32, kind="ExternalInput")
    lngT = nc.dram_tensor("lngT", [128, NDC], F32, kind="ExternalInput")
    lnbT = nc.dram_tensor("lnbT", [128, NDC], F32, kind="ExternalInput")
    outw = nc.dram_tensor("outw", [128, NDC, VSP], BF16, kind="ExternalInput")
    outbV = nc.dram_tensor("outbV", [1, VSP], BF16, kind="ExternalInput")
    probsO = nc.dram_tensor("probsO", [S, VSP], F32, kind="ExternalOutput")
    import os as _os
    DBG = bool(int(_os.environ.get("KB_DEBUG", "0")))
    if DBG:
        dbg_hT = nc.dram_tensor("dbg_hT", [128, NDC, S], BF16, kind="ExternalOutput")
        dbg_h1T = nc.dram_tensor("dbg_h1T", [128, NDC, S], BF16, kind="ExternalOutput")
        dbg_h2T = nc.dram_tensor("dbg_h2T", [128, NDC, S], BF16, kind="ExternalOutput")
        dbg_outT = nc.dram_tensor("dbg_outT", [128, NDC, S], BF16, kind="ExternalOutput")
        dbg_at1 = nc.dram_tensor("dbg_at1", [128, NSC, SC], BF16, kind="ExternalOutput")
        dbg_at2 = nc.dram_tensor("dbg_at2", [128, NSC, SC], BF16, kind="ExternalOutput")
        dbg_e = nc.dram_tensor("dbg_e", [128, VSP], BF16, kind="ExternalOutput")
        dbg_den = nc.dram_tensor("dbg_den", [128, NVC8], F32, kind="ExternalOutput")
        dbg_hpar = nc.dram_tensor("dbg_hpar", [D, S], BF16, kind="ExternalOutput")
        dbg_hred = nc.dram_tensor("dbg_hred", [D, S], BF16, kind="ExternalOutput")

    with tile.TileContext(nc) as tc:
        with tc.tile_pool(name="dram", bufs=1, space="DRAM") as dram, \
             tc.tile_pool(name="ps", bufs=3, space="PSUM") as ps, \
             tc.tile_pool(name="ps2", bufs=2, space="PSUM") as ps2, \
             tc.tile_pool(name="const", bufs=1) as const, \
             tc.tile_pool(name="ev", bufs=2) as evp, \
             tc.tile_pool(name="addin", bufs=3) as adp, \
             tc.tile_pool(name="addin2", bufs=1) as adp2, \
             tc.tile_pool(name="x2p", bufs=2) as x2p, \
             tc.tile_pool(name="bcast", bufs=2) as bcp, \
             tc.tile_pool(name="small", bufs=2) as smp:

            # internal DRAM (tracked pool tiles)
            h_par = [dram.tile([D, 2 * SC], BF16, tag=f"hp{p}", name=f"h_par{p}")
                     for p in range(2)]
            h_red = [dram.tile([D, 2 * SC], BF16, tag=f"hr{p}", addr_space="Shared",
                               name=f"h_red{p}") for p in range(2)]
            a_in = [[dram.tile([128, SC], BF16, tag=f"ai{m}_{sc}", name=f"a{m}_in{sc}")
                     for sc in range(NSC)] for m in (0, 1)]
            a_out = [[dram.tile([D, SC], BF16, tag=f"ao{m}_{sc}", addr_space="Shared",
                                name=f"a{m}_out{sc}") for sc in range(NSC)] for m in (0, 1)]
            y_par = [dram.tile([D, SC], BF16, tag=f"yp{sc}", name=f"y_par{sc}")
                     for sc in range(NSC)]
            y_red = [dram.tile([D, SC], BF16, tag=f"yr{sc}", addr_space="Shared",
                               name=f"y_red{sc}") for sc in range(NSC)]
            ss_in = [dram.tile([128, 1], F32, tag=f"si{tt}", name=f"ss_in{tt}")
                     for tt in range(NTT)]
            ss_out = [dram.tile([128, 1], F32, tag=f"so{tt}", addr_space="Shared",
                                name=f"ss_out{tt}") for tt in range(NTT)]

            # constants
            ones_bf_col = const.tile([128, 1], BF16, tag="c1")
            nc.vector.memset(ones_bf_col[:, :], 1.0)
            ones_row = const.tile([1, 128], F32, tag="c3")
            nc.vector.memset(ones_row[:, :], 1.0)
            ones_row64 = const.tile([1, 64], F32, tag="c4")
            nc.vector.memset(ones_row64[:, :], 1.0)
            ones_row128_bf = const.tile([1, 128], BF16, tag="c6")
            nc.vector.memset(ones_row128_bf[:, :], 1.0)
            eps_tile = const.tile([1, 1], F32, tag="c5")
            nc.vector.memset(eps_tile[:, :], 1e-5)
            ident_sb = const.tile([128, 128], BF16, tag="ident")
            nc.sync.dma_start(ident_sb[:, :], identT[:, :])
            bqk_sb = [const.tile([128, 2], F32, tag=f"bqk{m}", name=f"bqk_sb{m}") for m in range(2)]
            bv_sb = [const.tile([128, 1], F32, tag=f"bv{m}", name=f"bv_sb{m}") for m in range(2)]
            for m in range(2):
                nc.sync.dma_start(bqk_sb[m][:, :], bqk[m][:, :])
                nc.sync.dma_start(bv_sb[m][:, :], bv[m][:, :])
            f1b_sb = const.tile([128, NFC], F32, tag="f1b")
            nc.sync.dma_start(f1b_sb[:, :], f1b[:, :])
            f2bT_sb = const.tile([128, NDC], F32, tag="f2bT")
            nc.sync.dma_start(f2bT_sb[:, :], f2bT[:, :])
            lng_sb = const.tile([128, NDC], F32, tag="lng")
            nc.sync.dma_start(lng_sb[:, :], lngT[:, :])
            lnb_sb = const.tile([128, NDC], F32, tag="lnb")
            nc.sync.dma_start(lnb_sb[:, :], lnbT[:, :])

            # ---------- embedding GEMM:  hT_partial = embW^T @ xT ----------
            # sc-pairs of 1024 tokens; dc-pair groups (4 PSUM banks) so
            # eviction of one group overlaps the next; xT half resident.
            with tc.tile_pool(name="embw", bufs=1) as embp, \
                 tc.tile_pool(name="xt", bufs=1) as xtp:
                embw_sb = embp.tile([128, NKC, D], BF16, tag="embw")
                xthalf = xtp.tile([128, NKC, 2 * SC], BF16, tag="xt")
                for p in range(2):
                    for kc in range(NKC):
                        if p == 0:
                            nc.sync.dma_start(embw_sb[:, kc, :], embW[ts(kc, 128), :])
                        nc.sync.dma_start(
                            xthalf[:, kc, :], xT[ts(kc, 128), ts(p, 2 * SC)])
                    for dc in range(NDC):
                        pes = ps2.tile([128, 2, SC], F32, tag="ps2",
                                       name=f"pe_{p}_{dc}")
                        for kc in range(NKC):
                            for hh in range(2):
                                nc.tensor.matmul(
                                    pes[:, hh, :],
                                    embw_sb[:, kc, ts(dc, 128)],
                                    xthalf[:, kc, ts(hh, SC)],
                                    start=(kc == 0),
                                    stop=(kc == NKC - 1),
                                )
                        hv = evp.tile([128, 2 * SC], BF16, tag="ev")
                        nc.scalar.activation(hv[:, :], pes[:, :, :], AF.Copy)
                        nc.sync.dma_start(h_par[p][ts(dc, 128), :], hv[:, :])
                    nc.gpsimd.collective_compute(
                        "AllReduce", ALU.add, replica_groups=RG,
                        ins=[h_par[p][:, :].opt()], outs=[h_red[p][:, :].opt()],
                    )
                    if DBG:
                        nc.sync.dma_start(dbg_hpar[:, ts(p, 2 * SC)], h_par[p][:, :])
                        nc.sync.dma_start(dbg_hred[:, ts(p, 2 * SC)], h_red[p][:, :])

            # ======== transformer body (s-chunk pipelined) ========
            with tc.tile_pool(name="acts", bufs=2) as acts:
                # residual + layernorm over feature dim for ONE s-chunk.
                # residual sum written IN PLACE into prevT (dead after).
                def ln_sc(prevT, sc, addin_dram, newT, name, extra_bias=None):
                    scs = ts(sc, SC)
                    for dc in range(NDC):
                        ad = adp.tile([128, SC], BF16, tag="addin",
                                      name=f"ad_{name}_{dc}")
                        nc.sync.dma_start(ad[:, :], addin_dram[ts(dc, 128), :])
                        if extra_bias is not None:
                            ab = adp2.tile([128, SC], BF16, tag="addin2",
                                           name=f"ab_{name}_{dc}")
                            nc.scalar.activation(ab[:, :], ad[:, :], AF.Identity,
                                                 bias=extra_bias[:, dc:dc + 1])
                            nc.vector.tensor_add(prevT[:, dc, scs], prevT[:, dc, scs], ab[:, :])
                        else:
                            nc.vector.tensor_add(prevT[:, dc, scs], prevT[:, dc, scs], ad[:, :])
                    st = ps.tile([65, SC], F32, tag="ps", name=f"st_{name}")
                    for dc in range(NDC):
                        x2 = x2p.tile([128, SC], BF16, tag="x2", name=f"x2_{name}_{dc}")
                        nc.vector.tensor_mul(x2[:, :], prevT[:, dc, scs], prevT[:, dc, scs])
                        nc.tensor.matmul(st[0:1, :], ones_bf_col[:, :],
                                         prevT[:, dc, scs],
                                         start=(dc == 0), stop=(dc == NDC - 1))
                        nc.tensor.matmul(st[64:65, :], ones_bf_col[:, :],
                                         x2[:, :],
                                         start=(dc == 0), stop=(dc == NDC - 1))
                    nm = smp.tile([1, SC], BF16, tag="nm", name=f"nm_{name}")
                    nc.vector.tensor_scalar_mul(nm[:, :], st[0:1, :], -1.0 / D)
                    e2 = smp.tile([1, SC], F32, tag="e2", name=f"e2_{name}")
                    nc.vector.tensor_scalar_mul(e2[:, :], st[64:65, :], 1.0 / D)
                    musq = smp.tile([1, SC], F32, tag="musq", name=f"musq_{name}")
                    nc.vector.tensor_mul(musq[:, :], nm[:, :], nm[:, :])
                    nc.vector.tensor_sub(e2[:, :], e2[:, :], musq[:, :])
                    nc.scalar.activation(e2[:, :], e2[:, :], AF.Sqrt,
                                         bias=eps_tile[:, :])
                    inv = smp.tile([1, SC], BF16, tag="inv1", name=f"inv_{name}")
                    with nc.allow_low_precision(reason="bf16 LN scale is within tolerance"):
                        nc.vector.reciprocal(inv[:, :], e2[:, :])
                    negmu_b = bcp.tile([128, SC], BF16, tag="negmu", bufs=1,
                                       name=f"nmb_{name}")
                    inv_b = bcp.tile([128, SC], BF16, tag="inv", bufs=1,
                                     name=f"invb_{name}")
                    nc.gpsimd.partition_broadcast(negmu_b[:, :], nm[:, :])
                    nc.gpsimd.partition_broadcast(inv_b[:, :], inv[:, :])
                    for dc in range(NDC):
                        t1 = x2p.tile([128, SC], BF16, tag="t1b", name=f"t1_{name}_{dc}")
                        nc.vector.tensor_add(t1[:, :], prevT[:, dc, scs], negmu_b[:, :])
                        nc.vector.tensor_mul(t1[:, :], t1[:, :], inv_b[:, :])
                        nc.vector.tensor_scalar(newT[:, dc, scs], t1[:, :],
                                                lng_sb[:, dc:dc + 1],
                                                lnb_sb[:, dc:dc + 1],
                                                op0=ALU.mult, op1=ALU.add)

                hT = acts.tile([128, NDC, S], BF16, tag="act", name="hT")

                def build_hT(p):
                    for dc in range(NDC):
                        hr = adp.tile([128, 2 * SC], BF16, tag="hrad", bufs=2,
                                      name=f"hr_{p}_{dc}")
                        nc.sync.dma_start(hr[:, :], h_red[p][ts(dc, 128), :])
                        pb = adp.tile([128, 2 * SC], BF16, tag="pbad", bufs=2,
                                      name=f"pb_{p}_{dc}")
                        nc.sync.dma_start(pb[:, :], pebT[ts(dc, 128), ts(p, 2 * SC)])
                        nc.vector.tensor_add(hT[:, dc, ts(p, 2 * SC)], hr[:, :], pb[:, :])

                def qkv_sc(mi, actT, sc, qT2, kT2, V_sb, qkw_sb, vw_sb):
                    """Q/K projections + V~ tiles for one s-chunk.
                    V is computed weight-stationary ([dk2, t] in PSUM), then
                    PE-transposed per t-tile into the [t, V_h0|1|V_h1|1]
                    layout the AV matmul wants."""
                    scs = ts(sc, SC)
                    for wi, dst in ((0, qT2), (1, kT2)):
                        pq = ps.tile([128, SC], F32, tag="ps",
                                     name=f"pq{mi}_{wi}_{sc}")
                        for dc in range(NDC):
                            nc.tensor.matmul(
                                pq[:, :],
                                qkw_sb[:, dc, ts(wi, 128)],
                                actT[:, dc, scs],
                                start=(dc == 0), stop=(dc == NDC - 1),
                            )
                        nc.vector.tensor_scalar_add(
                            dst[:, sc, :], pq[:, :], bqk_sb[mi][:, wi:wi + 1])
                    pvt = ps.tile([128, SC], F32, tag="ps", name=f"pvt{mi}_{sc}")
                    for dc in range(NDC):
                        nc.tensor.matmul(
                            pvt[:, :], vw_sb[:, dc, :], actT[:, dc, scs],
                            start=(dc == 0), stop=(dc == NDC - 1),
                        )
                    VT_tmp = x2p.tile([128, SC], BF16, tag="vtt",
                                      name=f"vtt{mi}_{sc}")
                    nc.scalar.activation(VT_tmp[:, :], pvt[:, :], AF.Copy)
                    for ttl in range(4):
                        tt = 4 * sc + ttl
                        ptr = ps.tile([128, 128], BF16, tag="ptr", bufs=1,
                                      name=f"ptr{mi}_{tt}")
                        nc.tensor.transpose(ptr[:, :], VT_tmp[:, ts(ttl, 128)],
                                            ident_sb[:, :])
                        nc.vector.tensor_copy(V_sb[:, tt, 0:64], ptr[:, 0:64])
                        nc.vector.tensor_copy(V_sb[:, tt, 65:129], ptr[:, 64:128])
                        nc.vector.memset(V_sb[:, tt, 64:65], 1.0)
                        nc.vector.memset(V_sb[:, tt, 129:130], 1.0)

                def scores_sc(mi, sc, qT2, kT2, V_sb, mask_sb, attnT):
                    masked = mask_sb is not None
                    """Packed-head scores + AV for one s-chunk; writes
                    normalized attn (+bias) into attnT[:, sc-slice],
                    DMAs to a_in and AllGathers to a_out."""
                    po = [ps.tile([128, SC], F32, tag="ps", name=f"po{mi}_{h}_{sc}")
                          for h in range(2)]
                    tts = list(range(4 * (sc + 1))) if masked else list(range(NTT))
                    for i, tt in enumerate(tts):
                        pscr = ps2.tile([128, 2, SC], F32, tag="ps2",
                                        name=f"pscr{mi}_{sc}_{tt}")
                        for h in range(2):
                            nc.tensor.matmul(
                                pscr[:, h, :],
                                kT2[ts(h, 64), tt // 4, ts(tt % 4, 128)],
                                qT2[ts(h, 64), sc, :],
                                start=True, stop=True,
                            )
                        et = evp.tile([128, 2, SC], BF16, tag="exp")
                        nc.scalar.activation(et[:, :, :], pscr[:, :, :], AF.Exp,
                                             scale=1.0 / D)
                        if masked and tt >= 4 * sc:
                            for h in range(2):
                                nc.vector.tensor_mul(
                                    et[:, h, :], et[:, h, :],
                                    mask_sb[:, ts(tt - 4 * sc, SC)],
                                )
                        for h in range(2):
                            nc.tensor.matmul(
                                po[h][0:65, :],
                                V_sb[:, tt, ts(h, 65)],
                                et[:, h, :],
                                start=(i == 0), stop=(i == len(tts) - 1),
                            )
                    for h in range(2):
                        rec = smp.tile([1, SC], BF16, tag="rec", bufs=2,
                                       name=f"rec{mi}_{h}_{sc}")
                        with nc.allow_low_precision(reason="bf16 softmax scale is within tolerance"):
                            nc.vector.reciprocal(rec[:, :], po[h][64:65, :])
                        rb = bcp.tile([64, SC], BF16, tag="rb", bufs=2,
                                      name=f"rb{mi}_{h}_{sc}")
                        nc.gpsimd.partition_broadcast(rb[:, :], rec[:, :])
                        tmp = x2p.tile([128, SC], BF16, tag="t1b",
                                       name=f"avtmp{mi}_{h}_{sc}")
                        nc.vector.tensor_mul(tmp[0:64, :], po[h][0:64, :], rb[:, :])
                        nc.vector.tensor_scalar_add(
                            attnT[ts(h, 64), sc, :], tmp[0:64, :],
                            bv_sb[mi][ts(h, 64), :])
                    nc.sync.dma_start(a_in[mi][sc][:, :], attnT[:, sc, :])
                    nc.gpsimd.collective_compute(
                        "AllGather", ALU.bypass, replica_groups=RG,
                        ins=[a_in[mi][sc][:, :].opt()], outs=[a_out[mi][sc][:, :].opt()],
                    )

                # ---- MHA1 (masked), pipelined over sc as AR chunks land
                with tc.tile_pool(name="aw1", bufs=1) as aw1, \
                     tc.tile_pool(name="at1", bufs=1) as at1:
                    qkw1_sb = aw1.tile([128, NDC, 256], BF16, tag="qkw1")
                    nc.sync.dma_start(qkw1_sb[:, :, :], qkw[0][:, :, :])
                    vw1_sb = aw1.tile([128, NDC, 128], BF16, tag="vw1")
                    nc.sync.dma_start(vw1_sb[:, :, :], vw[0][:, :, :])
                    mask_sb = at1.tile([128, 4 * SC], BF16, tag="mask")
                    nc.sync.dma_start(mask_sb[:, :], maskT[:, :])
                    qT2_1 = at1.tile([128, NSC, SC], BF16, tag="qT1", name="qT2_1")
                    kT2_1 = at1.tile([128, NSC, SC], BF16, tag="kT1", name="kT2_1")
                    V_sb1 = at1.tile([128, NTT, 130], BF16, tag="V1", name="V_sb1")
                    attnT1 = at1.tile([128, NSC, SC], BF16, tag="atn1", name="attnT1")
                    for p in range(2):
                        build_hT(p)
                        for sc in (2 * p, 2 * p + 1):
                            qkv_sc(0, hT, sc, qT2_1, kT2_1, V_sb1, qkw1_sb, vw1_sb)
                            scores_sc(0, sc, qT2_1, kT2_1, V_sb1, mask_sb, attnT1)
                    if DBG:
                        nc.sync.dma_start(dbg_hT[:, :, :], hT[:, :, :])
                        nc.sync.dma_start(dbg_at1[:, :, :], attnT1[:, :, :])

                # ---- h1 = LN(h + attn1) per sc; QKV2 per sc; MHA2 scores
                h1T = acts.tile([128, NDC, S], BF16, tag="act", name="h1T")
                with tc.tile_pool(name="aw2", bufs=1) as aw2, \
                     tc.tile_pool(name="at2", bufs=1) as at2:
                    qkw2_sb = aw2.tile([128, NDC, 256], BF16, tag="qkw2")
                    nc.sync.dma_start(qkw2_sb[:, :, :], qkw[1][:, :, :])
                    vw2_sb = aw2.tile([128, NDC, 128], BF16, tag="vw2")
                    nc.sync.dma_start(vw2_sb[:, :, :], vw[1][:, :, :])
                    qT2_2 = at2.tile([128, NSC, SC], BF16, tag="qT2", name="qT2_2")
                    kT2_2 = at2.tile([128, NSC, SC], BF16, tag="kT2", name="kT2_2")
                    V_sb2 = at2.tile([128, NTT, 130], BF16, tag="V2", name="V_sb2")
                    attnT2 = at2.tile([128, NSC, SC], BF16, tag="atn2", name="attnT2")
                    for sc in range(NSC):
                        ln_sc(hT, sc, a_out[0][sc], h1T, f"h1_{sc}")
                        qkv_sc(1, h1T, sc, qT2_2, kT2_2, V_sb2, qkw2_sb, vw2_sb)
                    for sc in range(NSC):
                        scores_sc(1, sc, qT2_2, kT2_2, V_sb2, None, attnT2)
                    if DBG:
                        nc.sync.dma_start(dbg_h1T[:, :, :], h1T[:, :, :])
                        nc.sync.dma_start(dbg_at2[:, :, :], attnT2[:, :, :])

                # ---- h2 = LN(h1 + attn2) per sc; FFN per sc; ARy per sc
                h2T = acts.tile([128, NDC, S], BF16, tag="act", name="h2T")
                with tc.tile_pool(name="outwp", bufs=1) as owp:
                    # outw prefetch: DMAs overlap the whole FFN phase
                    outw_sb = owp.tile([128, NDC, VSP], BF16, tag="outw")
                    for dc in range(NDC):
                        nc.sync.dma_start(outw_sb[:, dc, :], outw[:, dc, :])
                    outb_sb = owp.tile([1, VSP], BF16, tag="outb")
                    nc.sync.dma_start(outb_sb[:, :], outbV[:, :])
                    with tc.tile_pool(name="ffw", bufs=1) as ffp:
                        f1w_sb = ffp.tile([128, NDC, FS], BF16, tag="f1w")
                        nc.sync.dma_start(f1w_sb[:, :, :], f1w[:, :, :])
                        f2w_sb = ffp.tile([128, NFC, D], BF16, tag="f2w")
                        nc.sync.dma_start(f2w_sb[:, :, :], f2w[:, :, :])
                        for sc in range(NSC):
                            ln_sc(h1T, sc, a_out[1][sc], h2T, f"h2_{sc}")
                            uT = ffp.tile([128, NFC, SC], BF16, tag="uT", bufs=2,
                                          name=f"uT_{sc}")
                            for fc in range(NFC):
                                pu = ps.tile([128, SC], F32, tag="ps", name=f"pu_{fc}_{sc}")
                                for dc in range(NDC):
                                    nc.tensor.matmul(pu[:, :], f1w_sb[:, dc, ts(fc, 128)],
                                                     h2T[:, dc, ts(sc, SC)],
                                                     start=(dc == 0), stop=(dc == NDC - 1))
                                nc.scalar.activation(uT[:, fc, :], pu[:, :], AF.Relu,
                                                     bias=f1b_sb[:, fc:fc + 1])
                            for dc in range(NDC):
                                py = ps.tile([128, SC], F32, tag="ps", name=f"py_{dc}_{sc}")
                                for fc in range(NFC):
                                    nc.tensor.matmul(py[:, :], f2w_sb[:, fc, ts(dc, 128)],
                                                     uT[:, fc, :],
                                                     start=(fc == 0), stop=(fc == NFC - 1))
                                yt = evp.tile([128, SC], BF16, tag="ev")
                                nc.scalar.activation(yt[:, :], py[:, :], AF.Copy)
                                nc.sync.dma_start(y_par[sc][ts(dc, 128), :], yt[:, :])
                            nc.gpsimd.collective_compute(
                                "AllReduce", ALU.add, replica_groups=RG,
                                ins=[y_par[sc][:, :].opt()], outs=[y_red[sc][:, :].opt()],
                            )

                    # ==== out = LN(h2+ffn); logits GEMM + fused softmax ====
                    # [token, vocab] layout: stationary outT tile reused over
                    # vocab chunks, exp eviction accumulates the denominator.
                    outT = acts.tile([128, NDC, S], BF16, tag="act", name="outT")
                    if DBG:
                        nc.sync.dma_start(dbg_h2T[:, :, :], h2T[:, :, :])
                    with tc.tile_pool(name="esb", bufs=2) as esp, \
                         tc.tile_pool(name="den", bufs=2) as denp, \
                         tc.tile_pool(name="pp", bufs=3) as ppp:
                        for sc in range(NSC):
                            ln_sc(h2T, sc, y_red[sc], outT, f"out_{sc}",
                                  extra_bias=f2bT_sb)
                            for ttl in range(4):
                                tt = 4 * sc + ttl
                                e_sb = esp.tile([128, VSP], BF16, tag="esb",
                                                name=f"esb_{tt}")
                                den = denp.tile([128, 4], F32, tag="den",
                                                name=f"den_{tt}")
                                den1 = denp.tile([128, 1], F32, tag="den1",
                                                 name=f"den1_{tt}")
                                for vq in range(4):      # vocab quarters: 2 banks
                                    pls = ps2.tile([128, 2, SC], F32, tag="ps2",
                                                   name=f"pl_{tt}_{vq}")
                                    for i in range(2):
                                        vc = vq * 2 + i
                                        nc.tensor.matmul(pls[:, i, :], ones_row128_bf[:, :],
                                                         outb_sb[:, ts(vc, SC)],
                                                         start=True, stop=False)
                                    for dc in range(NDC):
                                        for i in range(2):
                                            vc = vq * 2 + i
                                            nc.tensor.matmul(
                                                pls[:, i, :],
                                                outT[:, dc, ts(tt, 128)],
                                                outw_sb[:, dc, ts(vc, SC)],
                                                start=False, stop=(dc == NDC - 1),
                                            )
                                    nc.scalar.activation(
                                        e_sb[:, ts(vq, 2 * SC)], pls[:, :, :], AF.Exp,
                                        accum_out=den[:, vq:vq + 1],
                                    )
                                nc.vector.tensor_reduce(
                                    den1[:, :], den[:, :],
                                    axis=mybir.AxisListType.X, op=ALU.add,
                                )
                                if DBG and tt == 0:
                                    nc.sync.dma_start(dbg_e[:, :], e_sb[:, :])
                                    nc.sync.dma_start(dbg_den[:, :], den[:, :])
                                nc.sync.dma_start(ss_in[tt][:, :], den1[:, :])
                                nc.gpsimd.collective_compute(
                                    "AllReduce", ALU.add, replica_groups=RG,
                                    ins=[ss_in[tt][:, :].opt()], outs=[ss_out[tt][:, :].opt()],
                                )
                                ssum = denp.tile([128, 1], F32, tag="ssum",
                                                 name=f"ssum_{tt}")
                                nc.sync.dma_start(ssum[:, :], ss_out[tt][:, :])
                                srec = denp.tile([128, 1], F32, tag="srec",
                                                 name=f"srec_{tt}")
                                nc.vector.reciprocal(srec[:, :], ssum[:, :])
                                for vc in range(NVC8):
                                    pr = ppp.tile([128, SC], F32, tag="pp",
                                                  name=f"pp_{tt}_{vc}")
                                    nc.vector.tensor_scalar_mul(
                                        pr[:, :], e_sb[:, ts(vc, SC)],
                                        srec[:, 0:1])
                                    nc.sync.dma_start(
                                        probsO[ts(tt, 128), ts(vc, SC)], pr[:, :])
                        if DBG:
                            nc.sync.dma_start(dbg_outT[:, :, :], outT[:, :, :])

    nc.compile()
    return nc


def _positional_encoding():
    pos = np.arange(S, dtype=np.float32)[:, None]
    i = np.arange(0, D, 2, dtype=np.float32)
    ang = (pos * np.exp((-np.log(10000.0) * i / D).astype(np.float32))).astype(np.float32)
    pe = np.zeros((S, D), np.float32)
    pe[:, 0::2] = np.sin(ang)
    pe[:, 1::2] = np.cos(ang)
    return pe


def _bf(x):
    return np.ascontiguousarray(x).astype(ml_dtypes.bfloat16)


def _f32(x):
    return np.ascontiguousarray(x, dtype=np.float32)


def prepare_inputs(inp):
    """Full fp32 inputs -> per-core input maps (host-side sharding/layout)."""
    li = L - 1
    xT_full = np.ascontiguousarray(inp["x"].T)          # [V, S]
    peb = (inp["emb_b"][None, :] + _positional_encoding()).astype(np.float32)
    pebT = _bf(peb.T)                                    # [D, S]

    # causal mask patterns for the 4 diagonal t-tiles of an s-chunk
    t_loc = np.arange(128)[:, None]
    s_loc = np.arange(SC)[None, :]
    maskT = np.concatenate(
        [((p * 128 + t_loc) <= s_loc).astype(np.float32) for p in range(4)], axis=1
    )
    maskT = _bf(maskT)                                   # [128, 2048]

    in_maps = []
    for c in range(NCORES):
        m = {}
        xs = xT_full[c * VSR:(c + 1) * VSR]              # [4000, S]
        m["xT"] = _bf(np.concatenate([xs, np.zeros((VSP - VSR, S), np.float32)], 0))
        ew = inp["emb_W"][c * VSR:(c + 1) * VSR]
        m["embW"] = _bf(np.concatenate([ew, np.zeros((VSP - VSR, D), np.float32)], 0))
        m["pebT"] = pebT
        m["maskT"] = maskT
        m["identT"] = _bf(np.eye(128, dtype=np.float32))
        for mi, (Wq, bq, Wk, bk, Wv, bvv) in enumerate([
            (inp["Wq1"][li], inp["bq1"][li], inp["Wk1"][li], inp["bk1"][li],
             inp["Wv1"][li], inp["bv1"][li]),
            (inp["Wq2"][li], inp["bq2"][li], inp["Wk2"][li], inp["bk2"][li],
             inp["Wv2"][li], inp["bv2"][li]),
        ]):
            h0, h1 = 2 * c, 2 * c + 1
            qk = np.concatenate([Wq[h0], Wq[h1], Wk[h0], Wk[h1]], axis=1)  # [D, 256]
            m[f"qkw{mi+1}"] = _bf(qk.reshape(NDC, 128, 256).transpose(1, 0, 2))
            m[f"bqk{mi+1}"] = _f32(np.stack(
                [np.concatenate([bq[h0], bq[h1]]),
                 np.concatenate([bk[h0], bk[h1]])], axis=1))
            vp = np.concatenate([Wv[h0], Wv[h1]], axis=1)   # [D, 128]
            m[f"vw{mi+1}"] = _bf(vp.reshape(NDC, 128, 128).transpose(1, 0, 2))
            m[f"bv{mi+1}"] = _f32(np.concatenate([bvv[h0], bvv[h1]])[:, None])
        w1 = inp["ff_W1"][li][:, c * FS:(c + 1) * FS]    # [D, FS]
        m["f1w"] = _bf(w1.reshape(NDC, 128, FS).transpose(1, 0, 2))
        m["f1b"] = _f32(inp["ff_b1"][li][c * FS:(c + 1) * FS].reshape(NFC, 128).T)
        w2 = inp["ff_W2"][li][c * FS:(c + 1) * FS]       # [FS, D]
        m["f2w"] = _bf(w2.reshape(NFC, 128, D).transpose(1, 0, 2))
        m["f2bT"] = _f32(inp["ff_b2"][li].reshape(NDC, 128).T)
        m["lngT"] = _f32(inp["ln_g"].reshape(NDC, 128).T)
        m["lnbT"] = _f32(inp["ln_b"].reshape(NDC, 128).T)
        ow = inp["out_W"][:, c * VSR:(c + 1) * VSR]      # [D, 4000]
        ow = np.concatenate([ow, np.zeros((D, VSP - VSR), np.float32)], axis=1)
        m["outw"] = _bf(ow.reshape(NDC, 128, VSP).transpose(1, 0, 2))
        ob = np.full(VSP, -30.0, np.float32)
        ob[:VSR] = inp["out_b"][c * VSR:(c + 1) * VSR]
        m["outbV"] = _bf(ob[None, :])
        in_maps.append(m)
    return in_maps


_NC_CACHE = {}


def kernel(**inputs):
    inputs = {k: np.asarray(v, dtype=np.float32) for k, v in inputs.items()}
    if "nc" not in _NC_CACHE:
        _NC_CACHE["nc"] = build_bass()
    nc = _NC_CACHE["nc"]
    in_maps = prepare_inputs(inputs)
    import os
    trace = bool(int(os.environ.get("KB_TRACE", "0")))
    res = run_bass_kernel_spmd(nc, in_maps, list(range(NCORES)), trace=trace)
    LAST_RESULTS["res"] = res
    shards = [res.results[c]["probsO"][:, :VSR] for c in range(NCORES)]
    return np.ascontiguousarray(np.concatenate(shards, axis=1))


# revision 53
# speedup vs baseline: 1.5601x; 1.0575x over previous
"""Trainium2 Bass kernel for nn_LonelyDecoder (dense transformer, 8-core TP).

Key observations baked in:
 - In the reference, every layer recomputes from the embedding output `h`
   and only the LAST layer's `out` feeds the logits -> layers 0..L-2 are
   dead code. We compute: embedding GEMM, layer L-1, output GEMM+softmax.
 - Scores are tiny ((q.k)/1024, |s| < ~1), so softmax needs no max pass.
 - Activations in the transformer body are TRANSPOSED ([feature, seq]);
   the output GEMM flips to [seq, vocab] so the softmax denominator
   falls out of the ACT accumulator and the scale is per-partition.
 - Collectives are chunked (bf16) and overlapped under compute.

Sharding (8 cores):
 - vocab dim of x/emb_W/out_W (4000/core, padded to 4096)
 - heads of attention (2 heads/core), DFF of the FFN (512/core)
"""

import numpy as np
import ml_dtypes

import concourse.bacc as bacc
import concourse.bass as bass
import concourse.mybir as mybir
import concourse.tile as tile
from concourse.bass_utils import run_bass_kernel_spmd

F32 = mybir.dt.float32
BF16 = mybir.dt.bfloat16
AF = mybir.ActivationFunctionType
ALU = mybir.AluOpType

S, V, D, H, DK, DFF, L = 2048, 32000, 1024, 16, 64, 4096, 4
NCORES = 8
VSR = V // NCORES          # 4000 real vocab shard
VSP = 4096                 # padded vocab shard (32 x 128)
NKC = VSP // 128           # 32 vocab k-chunks
NVC8 = VSP // 512          # 8 vocab chunks of 512
NDC = D // 128             # 8 d-chunks
NSC = 4                    # s-chunks of 512
SC = 512
NTT = S // 128             # 16 t-tiles
FS = DFF // NCORES         # 512 ff shard
NFC = FS // 128            # 4 ff chunks
RG = [list(range(NCORES))]

LAST_RESULTS = {}          # stash for test harness (exec time etc.)


def ts(i, n):
    return slice(i * n, (i + 1) * n)


def build_bass():
    nc = bacc.Bacc(None, target_bir_lowering=False)

    # ---- I/O ----
    xT = nc.dram_tensor("xT", [VSP, S], BF16, kind="ExternalInput")
    embW = nc.dram_tensor("embW", [VSP, D], BF16, kind="ExternalInput")
    pebT = nc.dram_tensor("pebT", [D, S], BF16, kind="ExternalInput")
    qkw = [nc.dram_tensor(f"qkw{m}", [128, NDC, 256], BF16, kind="ExternalInput") for m in (1, 2)]
    bqk = [nc.dram_tensor(f"bqk{m}", [128, 2], F32, kind="ExternalInput") for m in (1, 2)]
    vw = [nc.dram_tensor(f"vw{m}", [128, NDC, 128], BF16, kind="ExternalInput") for m in (1, 2)]
    identT = nc.dram_tensor("identT", [128, 128], BF16, kind="ExternalInput")
    bv = [nc.dram_tensor(f"bv{m}", [128, 1], F32, kind="ExternalInput") for m in (1, 2)]
    maskT = nc.dram_tensor("maskT", [128, 4 * SC], BF16, kind="ExternalInput")
    f1w = nc.dram_tensor("f1w", [128, NDC, FS], BF16, kind="ExternalInput")
    f1b = nc.dram_tensor("f1b", [128, NFC], F32, kind="ExternalInput")
    f2w = nc.dram_tensor("f2w", [128, NFC, D], BF16, kind="ExternalInput")
    f2bT = nc.dram_tensor("f2bT", [128, NDC], F32, kind="ExternalInput")
    lngT = nc.dram_tensor("lngT", [128, NDC], F32, kind="ExternalInput")
    lnbT = nc.dram_tensor("lnbT", [128, NDC], F32, kind="ExternalInput")
    outw = nc.dram_tensor("outw", [128, NDC, VSP], BF16, kind="ExternalInput")
    outbV = nc.dram_tensor("outbV", [1, VSP], BF16, kind="ExternalInput")
    probsO = nc.dram_tensor("probsO", [S, VSP], F32, kind="ExternalOutput")
    import os as _os
    DBG = bool(int(_os.environ.get("KB_DEBUG", "0")))
    if DBG:
        dbg_hT = nc.dram_tensor("dbg_hT", [128, NDC, S], BF16, kind="ExternalOutput")
        dbg_h1T = nc.dram_tensor("dbg_h1T", [128, NDC, S], BF16, kind="ExternalOutput")
        dbg_h2T = nc.dram_tensor("dbg_h2T", [128, NDC, S], BF16, kind="ExternalOutput")
        dbg_outT = nc.dram_tensor("dbg_outT", [128, NDC, S], BF16, kind="ExternalOutput")
        dbg_at1 = nc.dram_tensor("dbg_at1", [128, NSC, SC], BF16, kind="ExternalOutput")
        dbg_at2 = nc.dram_tensor("dbg_at2", [128, NSC, SC], BF16, kind="ExternalOutput")
        dbg_e = nc.dram_tensor("dbg_e", [128, VSP], BF16, kind="ExternalOutput")
        dbg_den = nc.dram_tensor("dbg_den", [128, NVC8], F32, kind="ExternalOutput")
        dbg_hpar = nc.dram_tensor("dbg_hpar", [D, S], BF16, kind="ExternalOutput")
        dbg_hred = nc.dram_tensor("dbg_hred", [D, S], BF16, kind="ExternalOutput")

    with tile.TileContext(nc) as tc:
        with tc.tile_pool(name="dram", bufs=1, space="DRAM") as dram, \
             tc.tile_pool(name="ps", bufs=3, space="PSUM") as ps, \
             tc.tile_pool(name="ps2", bufs=2, space="PSUM") as ps2, \
             tc.tile_pool(name="const", bufs=1) as const, \
             tc.tile_pool(name="ev", bufs=2) as evp, \
             tc.tile_pool(name="addin", bufs=3) as adp, \
             tc.tile_pool(name="addin2", bufs=1) as adp2, \
             tc.tile_pool(name="x2p", bufs=2) as x2p, \
             tc.tile_pool(name="bcast", bufs=2) as bcp, \
             tc.tile_pool(name="small", bufs=2) as smp:

            # internal DRAM (tracked pool tiles)
            h_par = [dram.tile([D, 2 * SC], BF16, tag=f"hp{p}", name=f"h_par{p}")
                     for p in range(2)]
            h_red = [dram.tile([D, 2 * SC], BF16, tag=f"hr{p}", addr_space="Shared",
                               name=f"h_red{p}") for p in range(2)]
            a_in = [[dram.tile([128, SC], BF16, tag=f"ai{m}_{sc}", name=f"a{m}_in{sc}")
                     for sc in range(NSC)] for m in (0, 1)]
            a_out = [[dram.tile([D, SC], BF16, tag=f"ao{m}_{sc}", addr_space="Shared",
                                name=f"a{m}_out{sc}") for sc in range(NSC)] for m in (0, 1)]
            y_par = [dram.tile([D, SC], BF16, tag=f"yp{sc}", name=f"y_par{sc}")
                     for sc in range(NSC)]
            y_red = [dram.tile([D, SC], BF16, tag=f"yr{sc}", addr_space="Shared",
                               name=f"y_red{sc}") for sc in range(NSC)]
            ss_in = [dram.tile([128, 1], F32, tag=f"si{tt}", name=f"ss_in{tt}")
                     for tt in range(NTT)]
            ss_out = [dram.tile([128, 1], F32, tag=f"so{tt}", addr_space="Shared",
                                name=f"ss_out{tt}") for tt in range(NTT)]

            # constants
            ones_bf_col = const.tile([128, 1], BF16, tag="c1")
            nc.vector.memset(ones_bf_col[:, :], 1.0)
            ones_row = const.tile([1, 128], F32, tag="c3")
            nc.vector.memset(ones_row[:, :], 1.0)
            ones_row64 = const.tile([1, 64], F32, tag="c4")
            nc.vector.memset(ones_row64[:, :], 1.0)
            ones_row128_bf = const.tile([1, 128], BF16, tag="c6")
            nc.vector.memset(ones_row128_bf[:, :], 1.0)
            eps_tile = const.tile([1, 1], F32, tag="c5")
            nc.vector.memset(eps_tile[:, :], 1e-5)
            ident_sb = const.tile([128, 128], BF16, tag="ident")
            nc.sync.dma_start(ident_sb[:, :], identT[:, :])
            bqk_sb = [const.tile([128, 2], F32, tag=f"bqk{m}", name=f"bqk_sb{m}") for m in range(2)]
            bv_sb = [const.tile([128, 1], F32, tag=f"bv{m}", name=f"bv_sb{m}") for m in range(2)]
            for m in range(2):
                nc.sync.dma_start(bqk_sb[m][:, :], bqk[m][:, :])
                nc.sync.dma_start(bv_sb[m][:, :], bv[m][:, :])
            f1b_sb = const.tile([128, NFC], F32, tag="f1b")
            nc.sync.dma_start(f1b_sb[:, :], f1b[:, :])
            f2bT_sb = const.tile([128, NDC], F32, tag="f2bT")
            nc.sync.dma_start(f2bT_sb[:, :], f2bT[:, :])
            lng_sb = const.tile([128, NDC], F32, tag="lng")
            nc.sync.dma_start(lng_sb[:, :], lngT[:, :])
            lnb_sb = const.tile([128, NDC], F32, tag="lnb")
            nc.sync.dma_start(lnb_sb[:, :], lnbT[:, :])

            # ---------- embedding GEMM:  hT_partial = embW^T @ xT ----------
            # sc-pairs of 1024 tokens; dc-pair groups (4 PSUM banks) so
            # eviction of one group overlaps the next; xT half resident.
            with tc.tile_pool(name="embw", bufs=1) as embp, \
                 tc.tile_pool(name="xt", bufs=1) as xtp:
                embw_sb = embp.tile([128, NKC, D], BF16, tag="embw")
                xthalf = xtp.tile([128, NKC, 2 * SC], BF16, tag="xt")
                for p in range(2):
                    for kc in range(NKC):
                        if p == 0:
                            nc.sync.dma_start(embw_sb[:, kc, :], embW[ts(kc, 128), :])
                        nc.sync.dma_start(
                            xthalf[:, kc, :], xT[ts(kc, 128), ts(p, 2 * SC)])
                    for dc in range(NDC):
                        pes = ps2.tile([128, 2, SC], F32, tag="ps2",
                                       name=f"pe_{p}_{dc}")
                        for kc in range(NKC):
                            for hh in range(2):
                                nc.tensor.matmul(
                                    pes[:, hh, :],
                                    embw_sb[:, kc, ts(dc, 128)],
                                    xthalf[:, kc, ts(hh, SC)],
                                    start=(kc == 0),
                                    stop=(kc == NKC - 1),
                                )
                        hv = evp.tile([128, 2 * SC], BF16, tag="ev")
                        nc.scalar.activation(hv[:, :], pes[:, :, :], AF.Copy)
                        nc.sync.dma_start(h_par[p][ts(dc, 128), :], hv[:, :])
                    nc.gpsimd.collective_compute(
                        "AllReduce", ALU.add, replica_groups=RG,
                        ins=[h_par[p][:, :].opt()], outs=[h_red[p][:, :].opt()],
                    )
                    if DBG:
                        nc.sync.dma_start(dbg_hpar[:, ts(p, 2 * SC)], h_par[p][:, :])
                        nc.sync.dma_start(dbg_hred[:, ts(p, 2 * SC)], h_red[p][:, :])

            # ======== transformer body (s-chunk pipelined) ========
            with tc.tile_pool(name="acts", bufs=2) as acts:
                # residual + layernorm over feature dim for ONE s-chunk.
                # residual sum written IN PLACE into prevT (dead after).
                def ln_sc(prevT, sc, addin_dram, newT, name, extra_bias=None):
                    scs = ts(sc, SC)
                    for dc in range(NDC):
                        ad = adp.tile([128, SC], BF16, tag="addin",
                                      name=f"ad_{name}_{dc}")
                        nc.sync.dma_start(ad[:, :], addin_dram[ts(dc, 128), :])
                        if extra_bias is not None:
                            ab = adp2.tile([128, SC], BF16, tag="addin2",
                                           name=f"ab_{name}_{dc}")
                            nc.scalar.activation(ab[:, :], ad[:, :], AF.Identity,
                                                 bias=extra_bias[:, dc:dc + 1])
                            nc.vector.tensor_add(prevT[:, dc, scs], prevT[:, dc, scs], ab[:, :])
                        else:
                            nc.vector.tensor_add(prevT[:, dc, scs], prevT[:, dc, scs], ad[:, :])
                    st = ps.tile([65, SC], F32, tag="ps", name=f"st_{name}")
                    for dc in range(NDC):
                        x2 = x2p.tile([128, SC], BF16, tag="x2", name=f"x2_{name}_{dc}")
                        nc.vector.tensor_mul(x2[:, :], prevT[:, dc, scs], prevT[:, dc, scs])
                        nc.tensor.matmul(st[0:1, :], ones_bf_col[:, :],
                                         prevT[:, dc, scs],
                                         start=(dc == 0), stop=(dc == NDC - 1))
                        nc.tensor.matmul(st[64:65, :], ones_bf_col[:, :],
                                         x2[:, :],
                                         start=(dc == 0), stop=(dc == NDC - 1))
                    nm = smp.tile([1, SC], BF16, tag="nm", name=f"nm_{name}")
                    nc.vector.tensor_scalar_mul(nm[:, :], st[0:1, :], -1.0 / D)
                    e2 = smp.tile([1, SC], F32, tag="e2", name=f"e2_{name}")
                    nc.vector.tensor_scalar_mul(e2[:, :], st[64:65, :], 1.0 / D)
                    musq = smp.tile([1, SC], F32, tag="fr32", name=f"musq_{name}")
                    nc.vector.tensor_mul(musq[:, :], nm[:, :], nm[:, :])
                    nc.vector.tensor_sub(e2[:, :], e2[:, :], musq[:, :])
                    nc.scalar.activation(e2[:, :], e2[:, :], AF.Sqrt,
                                         bias=eps_tile[:, :])
                    invf = smp.tile([1, SC], F32, tag="fr32", name=f"invf_{name}")
                    nc.vector.reciprocal_approx_fast(invf[:, :], e2[:, :])
                    inv = smp.tile([1, SC], BF16, tag="inv1", name=f"inv_{name}")
                    nc.vector.tensor_copy(inv[:, :], invf[:, :])
                    negmu_b = bcp.tile([128, SC], BF16, tag="negmu", bufs=1,
                                       name=f"nmb_{name}")
                    inv_b = bcp.tile([128, SC], BF16, tag="inv", bufs=1,
                                     name=f"invb_{name}")
                    nc.gpsimd.partition_broadcast(negmu_b[:, :], nm[:, :])
                    nc.gpsimd.partition_broadcast(inv_b[:, :], inv[:, :])
                    for dc in range(NDC):
                        t1 = x2p.tile([128, SC], BF16, tag="t1b", name=f"t1_{name}_{dc}")
                        nc.vector.tensor_add(t1[:, :], prevT[:, dc, scs], negmu_b[:, :])
                        nc.vector.tensor_mul(t1[:, :], t1[:, :], inv_b[:, :])
                        nc.vector.tensor_scalar(newT[:, dc, scs], t1[:, :],
                                                lng_sb[:, dc:dc + 1],
                                                lnb_sb[:, dc:dc + 1],
                                                op0=ALU.mult, op1=ALU.add)

                hT = acts.tile([128, NDC, S], BF16, tag="act", name="hT")

                def build_hT(p, pool):
                    for dc in range(NDC):
                        hr = pool.tile([128, 2 * SC], BF16, tag="hrad", bufs=2,
                                       name=f"hr_{p}_{dc}")
                        nc.sync.dma_start(hr[:, :], h_red[p][ts(dc, 128), :])
                        pb = pool.tile([128, 2 * SC], BF16, tag="pbad", bufs=2,
                                       name=f"pb_{p}_{dc}")
                        nc.sync.dma_start(pb[:, :], pebT[ts(dc, 128), ts(p, 2 * SC)])
                        nc.vector.tensor_add(hT[:, dc, ts(p, 2 * SC)], hr[:, :], pb[:, :])

                def qkv_sc(mi, actT, sc, qT2, kT2, V_sb, qkw_sb, vw_sb):
                    """Q/K projections + V~ tiles for one s-chunk.
                    V is computed weight-stationary ([dk2, t] in PSUM), then
                    PE-transposed per t-tile into the [t, V_h0|1|V_h1|1]
                    layout the AV matmul wants."""
                    scs = ts(sc, SC)
                    for wi, dst in ((0, qT2), (1, kT2)):
                        pq = ps.tile([128, SC], F32, tag="ps",
                                     name=f"pq{mi}_{wi}_{sc}")
                        for dc in range(NDC):
                            nc.tensor.matmul(
                                pq[:, :],
                                qkw_sb[:, dc, ts(wi, 128)],
                                actT[:, dc, scs],
                                start=(dc == 0), stop=(dc == NDC - 1),
                            )
                        nc.vector.tensor_scalar_add(
                            dst[:, sc, :], pq[:, :], bqk_sb[mi][:, wi:wi + 1])
                    pvt = ps.tile([128, SC], F32, tag="ps", name=f"pvt{mi}_{sc}")
                    for dc in range(NDC):
                        nc.tensor.matmul(
                            pvt[:, :], vw_sb[:, dc, :], actT[:, dc, scs],
                            start=(dc == 0), stop=(dc == NDC - 1),
                        )
                    VT_tmp = x2p.tile([128, SC], BF16, tag="vtt",
                                      name=f"vtt{mi}_{sc}")
                    nc.scalar.activation(VT_tmp[:, :], pvt[:, :], AF.Copy)
                    for ttl in range(4):
                        tt = 4 * sc + ttl
                        ptr = ps.tile([128, 128], BF16, tag="ptr", bufs=1,
                                      name=f"ptr{mi}_{tt}")
                        nc.tensor.transpose(ptr[:, :], VT_tmp[:, ts(ttl, 128)],
                                            ident_sb[:, :])
                        nc.vector.tensor_copy(V_sb[:, tt, 0:64], ptr[:, 0:64])
                        nc.vector.tensor_copy(V_sb[:, tt, 65:129], ptr[:, 64:128])
                        nc.vector.memset(V_sb[:, tt, 64:65], 1.0)
                        nc.vector.memset(V_sb[:, tt, 129:130], 1.0)

                def scores_sc(mi, sc, qT2, kT2, V_sb, mask_sb, attnT):
                    masked = mask_sb is not None
                    """Packed-head scores + AV for one s-chunk; writes
                    normalized attn (+bias) into attnT[:, sc-slice],
                    DMAs to a_in and AllGathers to a_out."""
                    po = [ps.tile([128, SC], F32, tag="ps", name=f"po{mi}_{h}_{sc}")
                          for h in range(2)]
                    tts = list(range(4 * (sc + 1))) if masked else list(range(NTT))
                    for i, tt in enumerate(tts):
                        pscr = ps2.tile([128, 2, SC], F32, tag="ps2",
                                        name=f"pscr{mi}_{sc}_{tt}")
                        for h in range(2):
                            nc.tensor.matmul(
                                pscr[:, h, :],
                                kT2[ts(h, 64), tt // 4, ts(tt % 4, 128)],
                                qT2[ts(h, 64), sc, :],
                                start=True, stop=True,
                            )
                        et = evp.tile([128, 2, SC], BF16, tag="exp")
                        nc.scalar.activation(et[:, :, :], pscr[:, :, :], AF.Exp,
                                             scale=1.0 / D)
                        if masked and tt >= 4 * sc:
                            for h in range(2):
                                nc.vector.tensor_mul(
                                    et[:, h, :], et[:, h, :],
                                    mask_sb[:, ts(tt - 4 * sc, SC)],
                                )
                        for h in range(2):
                            nc.tensor.matmul(
                                po[h][0:65, :],
                                V_sb[:, tt, ts(h, 65)],
                                et[:, h, :],
                                start=(i == 0), stop=(i == len(tts) - 1),
                            )
                    for h in range(2):
                        dens = smp.tile([1, SC], F32, tag="fr32", bufs=2,
                                        name=f"dens{mi}_{h}_{sc}")
                        nc.vector.tensor_copy(dens[:, :], po[h][64:65, :])
                        recf = smp.tile([1, SC], F32, tag="fr32", bufs=2,
                                        name=f"recf{mi}_{h}_{sc}")
                        nc.vector.reciprocal_approx_fast(recf[:, :], dens[:, :])
                        rec = smp.tile([1, SC], BF16, tag="rec", bufs=2,
                                       name=f"rec{mi}_{h}_{sc}")
                        nc.vector.tensor_copy(rec[:, :], recf[:, :])
                        rb = bcp.tile([64, SC], BF16, tag="rb", bufs=2,
                                      name=f"rb{mi}_{h}_{sc}")
                        nc.gpsimd.partition_broadcast(rb[:, :], rec[:, :])
                        tmp = x2p.tile([128, SC], BF16, tag="t1b",
                                       name=f"avtmp{mi}_{h}_{sc}")
                        nc.vector.tensor_mul(tmp[0:64, :], po[h][0:64, :], rb[:, :])
                        nc.vector.tensor_scalar_add(
                            attnT[ts(h, 64), sc, :], tmp[0:64, :],
                            bv_sb[mi][ts(h, 64), :])
                    nc.sync.dma_start(a_in[mi][sc][:, :], attnT[:, sc, :])
                    nc.gpsimd.collective_compute(
                        "AllGather", ALU.bypass, replica_groups=RG,
                        ins=[a_in[mi][sc][:, :].opt()], outs=[a_out[mi][sc][:, :].opt()],
                    )

                # ---- MHA1 (masked), pipelined over sc as AR chunks land
                with tc.tile_pool(name="aw1", bufs=1) as aw1, \
                     tc.tile_pool(name="at1", bufs=1) as at1:
                    qkw1_sb = aw1.tile([128, NDC, 256], BF16, tag="qkw1")
                    nc.sync.dma_start(qkw1_sb[:, :, :], qkw[0][:, :, :])
                    vw1_sb = aw1.tile([128, NDC, 128], BF16, tag="vw1")
                    nc.sync.dma_start(vw1_sb[:, :, :], vw[0][:, :, :])
                    mask_sb = at1.tile([128, 4 * SC], BF16, tag="mask")
                    nc.sync.dma_start(mask_sb[:, :], maskT[:, :])
                    qT2_1 = at1.tile([128, NSC, SC], BF16, tag="qT1", name="qT2_1")
                    kT2_1 = at1.tile([128, NSC, SC], BF16, tag="kT1", name="kT2_1")
                    V_sb1 = at1.tile([128, NTT, 130], BF16, tag="V1", name="V_sb1")
                    attnT1 = at1.tile([128, NSC, SC], BF16, tag="atn1", name="attnT1")
                    for p in range(2):
                        build_hT(p, at1)
                        for sc in (2 * p, 2 * p + 1):
                            qkv_sc(0, hT, sc, qT2_1, kT2_1, V_sb1, qkw1_sb, vw1_sb)
                        for sc in (2 * p, 2 * p + 1):
                            scores_sc(0, sc, qT2_1, kT2_1, V_sb1, mask_sb, attnT1)
                    if DBG:
                        nc.sync.dma_start(dbg_hT[:, :, :], hT[:, :, :])
                        nc.sync.dma_start(dbg_at1[:, :, :], attnT1[:, :, :])

                # ---- h1 = LN(h + attn1) per sc; QKV2 per sc; MHA2 scores
                h1T = acts.tile([128, NDC, S], BF16, tag="act", name="h1T")
                with tc.tile_pool(name="aw2", bufs=1) as aw2, \
                     tc.tile_pool(name="at2", bufs=1) as at2:
                    qkw2_sb = aw2.tile([128, NDC, 256], BF16, tag="qkw2")
                    nc.sync.dma_start(qkw2_sb[:, :, :], qkw[1][:, :, :])
                    vw2_sb = aw2.tile([128, NDC, 128], BF16, tag="vw2")
                    nc.sync.dma_start(vw2_sb[:, :, :], vw[1][:, :, :])
                    qT2_2 = at2.tile([128, NSC, SC], BF16, tag="qT2", name="qT2_2")
                    kT2_2 = at2.tile([128, NSC, SC], BF16, tag="kT2", name="kT2_2")
                    V_sb2 = at2.tile([128, NTT, 130], BF16, tag="V2", name="V_sb2")
                    attnT2 = at2.tile([128, NSC, SC], BF16, tag="atn2", name="attnT2")
                    for sc in range(NSC):
                        ln_sc(hT, sc, a_out[0][sc], h1T, f"h1_{sc}")
                    for sc in range(NSC):
                        qkv_sc(1, h1T, sc, qT2_2, kT2_2, V_sb2, qkw2_sb, vw2_sb)
                    for sc in range(NSC):
                        scores_sc(1, sc, qT2_2, kT2_2, V_sb2, None, attnT2)
                    if DBG:
                        nc.sync.dma_start(dbg_h1T[:, :, :], h1T[:, :, :])
                        nc.sync.dma_start(dbg_at2[:, :, :], attnT2[:, :, :])

                # ---- h2 = LN(h1 + attn2) per sc; FFN per sc; ARy per sc
                h2T = acts.tile([128, NDC, S], BF16, tag="act", name="h2T")
                with tc.tile_pool(name="outwp", bufs=1) as owp:
                    # outw prefetch: DMAs overlap the whole FFN phase
                    outw_sb = owp.tile([128, NDC, VSP], BF16, tag="outw")
                    for dc in range(NDC):
                        nc.sync.dma_start(outw_sb[:, dc, :], outw[:, dc, :])
                    outb_sb = owp.tile([1, VSP], BF16, tag="outb")
                    nc.sync.dma_start(outb_sb[:, :], outbV[:, :])
                    with tc.tile_pool(name="ffw", bufs=1) as ffp:
                        f1w_sb = ffp.tile([128, NDC, FS], BF16, tag="f1w")
                        nc.sync.dma_start(f1w_sb[:, :, :], f1w[:, :, :])
                        f2w_sb = ffp.tile([128, NFC, D], BF16, tag="f2w")
                        nc.sync.dma_start(f2w_sb[:, :, :], f2w[:, :, :])
                        for sc in range(NSC):
                            ln_sc(h1T, sc, a_out[1][sc], h2T, f"h2_{sc}")
                        for sc in range(NSC):
                            uT = ffp.tile([128, NFC, SC], BF16, tag="uT", bufs=2,
                                          name=f"uT_{sc}")
                            for fc in range(NFC):
                                pu = ps.tile([128, SC], F32, tag="ps", name=f"pu_{fc}_{sc}")
                                for dc in range(NDC):
                                    nc.tensor.matmul(pu[:, :], f1w_sb[:, dc, ts(fc, 128)],
                                                     h2T[:, dc, ts(sc, SC)],
                                                     start=(dc == 0), stop=(dc == NDC - 1))
                                nc.scalar.activation(uT[:, fc, :], pu[:, :], AF.Relu,
                                                     bias=f1b_sb[:, fc:fc + 1])
                            for dc in range(NDC):
                                py = ps.tile([128, SC], F32, tag="ps", name=f"py_{dc}_{sc}")
                                for fc in range(NFC):
                                    nc.tensor.matmul(py[:, :], f2w_sb[:, fc, ts(dc, 128)],
                                                     uT[:, fc, :],
                                                     start=(fc == 0), stop=(fc == NFC - 1))
                                yt = evp.tile([128, SC], BF16, tag="ev")
                                nc.scalar.activation(yt[:, :], py[:, :], AF.Copy)
                                nc.sync.dma_start(y_par[sc][ts(dc, 128), :], yt[:, :])
                            nc.gpsimd.collective_compute(
                                "AllReduce", ALU.add, replica_groups=RG,
                                ins=[y_par[sc][:, :].opt()], outs=[y_red[sc][:, :].opt()],
                            )

                    # ==== out = LN(h2+ffn); logits GEMM + fused softmax ====
                    # [token, vocab] layout: stationary outT tile reused over
                    # vocab chunks, exp eviction accumulates the denominator.
                    outT = acts.tile([128, NDC, S], BF16, tag="act", name="outT")
                    if DBG:
                        nc.sync.dma_start(dbg_h2T[:, :, :], h2T[:, :, :])
                    with tc.tile_pool(name="esb", bufs=3) as esp, \
                         tc.tile_pool(name="den", bufs=2) as denp, \
                         tc.tile_pool(name="pp", bufs=3) as ppp:
                        for sc in range(NSC):
                            ln_sc(h2T, sc, y_red[sc], outT, f"out_{sc}",
                                  extra_bias=f2bT_sb)
                            for ttl in range(4):
                                tt = 4 * sc + ttl
                                e_sb = esp.tile([128, VSP], BF16, tag="esb",
                                                name=f"esb_{tt}")
                                den = denp.tile([128, 4], F32, tag="den",
                                                name=f"den_{tt}")
                                den1 = denp.tile([128, 1], F32, tag="den1",
                                                 name=f"den1_{tt}")
                                for vq in range(4):      # vocab quarters: 2 banks
                                    pls = ps2.tile([128, 2, SC], F32, tag="ps2",
                                                   name=f"pl_{tt}_{vq}")
                                    for i in range(2):
                                        vc = vq * 2 + i
                                        nc.tensor.matmul(pls[:, i, :], ones_row128_bf[:, :],
                                                         outb_sb[:, ts(vc, SC)],
                                                         start=True, stop=False)
                                    for dc in range(NDC):
                                        for i in range(2):
                                            vc = vq * 2 + i
                                            nc.tensor.matmul(
                                                pls[:, i, :],
                                                outT[:, dc, ts(tt, 128)],
                                                outw_sb[:, dc, ts(vc, SC)],
                                                start=False, stop=(dc == NDC - 1),
                                            )
                                    nc.scalar.activation(
                                        e_sb[:, ts(vq, 2 * SC)], pls[:, :, :], AF.Exp,
                                        accum_out=den[:, vq:vq + 1],
                                    )
                                nc.vector.tensor_reduce(
                                    den1[:, :], den[:, :],
                                    axis=mybir.AxisListType.X, op=ALU.add,
                                )
                                if DBG and tt == 0:
                                    nc.sync.dma_start(dbg_e[:, :], e_sb[:, :])
                                    nc.sync.dma_start(dbg_den[:, :], den[:, :])
                                nc.sync.dma_start(ss_in[tt][:, :], den1[:, :])
                                nc.gpsimd.collective_compute(
                                    "AllReduce", ALU.add, replica_groups=RG,
                                    ins=[ss_in[tt][:, :].opt()], outs=[ss_out[tt][:, :].opt()],
                                )
                                ssum = denp.tile([128, 1], F32, tag="ssum",
                                                 name=f"ssum_{tt}")
                                nc.sync.dma_start(ssum[:, :], ss_out[tt][:, :])
                                srec = denp.tile([128, 1], F32, tag="srec",
                                                 name=f"srec_{tt}")
                                nc.vector.reciprocal(srec[:, :], ssum[:, :])
                                for vc in range(NVC8):
                                    pr = ppp.tile([128, SC], F32, tag="pp",
                                                  name=f"pp_{tt}_{vc}")
                                    nc.vector.tensor_scalar_mul(
                                        pr[:, :], e_sb[:, ts(vc, SC)],
                                        srec[:, 0:1])
                                    nc.sync.dma_start(
                                        probsO[ts(tt, 128), ts(vc, SC)], pr[:, :])
                        if DBG:
                            nc.sync.dma_start(dbg_outT[:, :, :], outT[:, :, :])

    nc.compile()
    return nc


def _positional_encoding():
    pos = np.arange(S, dtype=np.float32)[:, None]
    i = np.arange(0, D, 2, dtype=np.float32)
    ang = (pos * np.exp((-np.log(10000.0) * i / D).astype(np.float32))).astype(np.float32)
    pe = np.zeros((S, D), np.float32)
    pe[:, 0::2] = np.sin(ang)
    pe[:, 1::2] = np.cos(ang)
    return pe


def _bf(x):
    return np.ascontiguousarray(x).astype(ml_dtypes.bfloat16)


def _f32(x):
    return np.ascontiguousarray(x, dtype=np.float32)


def prepare_inputs(inp):
    """Full fp32 inputs -> per-core input maps (host-side sharding/layout)."""
    li = L - 1
    xT_full = np.ascontiguousarray(inp["x"].T)          # [V, S]
    peb = (inp["emb_b"][None, :] + _positional_encoding()).astype(np.float32)
    pebT = _bf(peb.T)                                    # [D, S]

    # causal mask patterns for the 4 diagonal t-tiles of an s-chunk
    t_loc = np.arange(128)[:, None]
    s_loc = np.arange(SC)[None, :]
    maskT = np.concatenate(
        [((p * 128 + t_loc) <= s_loc).astype(np.float32) for p in range(4)], axis=1
    )
    maskT = _bf(maskT)                                   # [128, 2048]

    in_maps = []
    for c in range(NCORES):
        m = {}
        xs = xT_full[c * VSR:(c + 1) * VSR]              # [4000, S]
        m["xT"] = _bf(np.concatenate([xs, np.zeros((VSP - VSR, S), np.float32)], 0))
        ew = inp["emb_W"][c * VSR:(c + 1) * VSR]
        m["embW"] = _bf(np.concatenate([ew, np.zeros((VSP - VSR, D), np.float32)], 0))
        m["pebT"] = pebT
        m["maskT"] = maskT
        m["identT"] = _bf(np.eye(128, dtype=np.float32))
        for mi, (Wq, bq, Wk, bk, Wv, bvv) in enumerate([
            (inp["Wq1"][li], inp["bq1"][li], inp["Wk1"][li], inp["bk1"][li],
             inp["Wv1"][li], inp["bv1"][li]),
            (inp["Wq2"][li], inp["bq2"][li], inp["Wk2"][li], inp["bk2"][li],
             inp["Wv2"][li], inp["bv2"][li]),
        ]):
            h0, h1 = 2 * c, 2 * c + 1
            qk = np.concatenate([Wq[h0], Wq[h1], Wk[h0], Wk[h1]], axis=1)  # [D, 256]
            m[f"qkw{mi+1}"] = _bf(qk.reshape(NDC, 128, 256).transpose(1, 0, 2))
            m[f"bqk{mi+1}"] = _f32(np.stack(
                [np.concatenate([bq[h0], bq[h1]]),
                 np.concatenate([bk[h0], bk[h1]])], axis=1))
            vp = np.concatenate([Wv[h0], Wv[h1]], axis=1)   # [D, 128]
            m[f"vw{mi+1}"] = _bf(vp.reshape(NDC, 128, 128).transpose(1, 0, 2))
            m[f"bv{mi+1}"] = _f32(np.concatenate([bvv[h0], bvv[h1]])[:, None])
        w1 = inp["ff_W1"][li][:, c * FS:(c + 1) * FS]    # [D, FS]
        m["f1w"] = _bf(w1.reshape(NDC, 128, FS).transpose(1, 0, 2))
        m["f1b"] = _f32(inp["ff_b1"][li][c * FS:(c + 1) * FS].reshape(NFC, 128).T)
        w2 = inp["ff_W2"][li][c * FS:(c + 1) * FS]       # [FS, D]
        m["f2w"] = _bf(w2.reshape(NFC, 128, D).transpose(1, 0, 2))
        m["f2bT"] = _f32(inp["ff_b2"][li].reshape(NDC, 128).T)
        m["lngT"] = _f32(inp["ln_g"].reshape(NDC, 128).T)
        m["lnbT"] = _f32(inp["ln_b"].reshape(NDC, 128).T)
        ow = inp["out_W"][:, c * VSR:(c + 1) * VSR]      # [D, 4000]
        ow = np.concatenate([ow, np.zeros((D, VSP - VSR), np.float32)], axis=1)
        m["outw"] = _bf(ow.reshape(NDC, 128, VSP).transpose(1, 0, 2))
        ob = np.full(VSP, -30.0, np.float32)
        ob[:VSR] = inp["out_b"][c * VSR:(c + 1) * VSR]
        m["outbV"] = _bf(ob[None, :])
        in_maps.append(m)
    return in_maps


_NC_CACHE = {}


def kernel(**inputs):
    inputs = {k: np.asarray(v, dtype=np.float32) for k, v in inputs.items()}
    if "nc" not in _NC_CACHE:
        _NC_CACHE["nc"] = build_bass()
    nc = _NC_CACHE["nc"]
    in_maps = prepare_inputs(inputs)
    import os
    trace = bool(int(os.environ.get("KB_TRACE", "0")))
    res = run_bass_kernel_spmd(nc, in_maps, list(range(NCORES)), trace=trace)
    LAST_RESULTS["res"] = res
    shards = [res.results[c]["probsO"][:, :VSR] for c in range(NCORES)]
    return np.ascontiguousarray(np.concatenate(shards, axis=1))
